# revision 32
# baseline (speedup 1.0000x reference)
"""AdaMix forward on 8 Trainium2 NeuronCores (Bass/Tile), pure data parallel.

Per core: 8 samples. Pipeline per 4-sample group:
  1. dice loss (classes-on-partitions layout; matmul class sums, fused
     AMR/STT accumulation) -> per-sample mix count k and sort sign.
  2. per-patch conf sums (segmented reduce + 2-stage matmuls), ranks via
     all-pairs compares, o<->a pairing via one-hot matmuls -> per-sample
     gather index over 128 patch slots (o patches 0..63, a 64..127).
  3. patch mix via gpsimd ap_gather per channel chunk; strided DMAs keep
     1KB contiguous runs on both sides.
"""

import os
import numpy as np

import concourse.bass as bass
import concourse.mybir as mybir
import concourse.tile as tile
from concourse import bacc
from concourse.bass_utils import run_bass_kernel_spmd

F32 = mybir.dt.float32
I32 = mybir.dt.int32
I16 = mybir.dt.int16
AL = mybir.AluOpType
AF = mybir.ActivationFunctionType

N_CORES = 8
B = 64
SPB = B // N_CORES      # samples per core
GS = 4                  # samples per group
NG = SPB // GS          # groups per core
NCLS = 4
IMG = 256
PS = 32                 # patch side
NH = 8                  # patches per image side
NP = 64                 # patches per image
AGE_DIV = 1.0 + 1e-5

# ---------------- constant pack (one [128, CW] f32 tensor) ----------------
CO_CLS = 0      # [128,32]  cls_ones:  k%32==m   (class sum over c)
CO_BC32 = 32    # [64,128]  bc32:      m%32==k%32 (broadcast ph -> (c,ph), x2 rows)
CO_CCOL = 160   # [128,1]   c_col:     k//32
CO_RG8 = 161    # [128,16]  rgrp8:     k//8==m
CO_RG4 = 177    # [16,4]    rgrp4:     k//4==m
CO_ONES = 181   # [64,64]   ones
CO_QIOTA = 245  # [64,1]    qiota:     k
CO_PIOTA = 246  # [1,64]    piota:     m
CO_WMASK = 310  # [64,4]    wrapmask:  k//16==m
CO_WSEL = 314   # [64,16]   wrapsel:   k%16==m
CO_ID4 = 330    # [4,4]     identity
CO_ID1 = 334    # [1,1]     identity
CO_CGRP = 335   # [128,4]   cls_grp:   k//32==m
CO_BLK = 339    # [4,256]   blkmask:   k==m//64
CO_WM4 = 595    # [64,16]   wmask4:    k//16==m%4
CO_SMASK = 611  # [128,16]  smask:     k//32==m//4
CO_PISEL = 627  # [16,128]  pisel:     k==m%16
CW = 756

_exec_info = {}


def _install_ntff_hook():
    """The agent image's antenv lacks axon_hooks; rebuild it from the boot
    helpers so trace=True (BASS_TRACE=1) works for profiling."""
    import sys
    import types
    try:
        import antenv.axon_hooks  # noqa: F401
        return
    except ImportError:
        pass
    try:
        import antenv
        from trn_agent_boot.trn_boot import _ntff_profile_via_ctypes
        hook = _ntff_profile_via_ctypes("/opt/axon/libaxon_pjrt.so")
        mod = types.ModuleType("antenv.axon_hooks")
        state = {"hook": hook}
        mod.set_axon_ntff_profile_hook = lambda h: state.update(hook=h)
        mod.get_axon_ntff_profile_hook = lambda: state["hook"]
        sys.modules["antenv.axon_hooks"] = mod
        antenv.axon_hooks = mod
    except Exception:
        pass


_install_ntff_hook()


def _build_consts() -> np.ndarray:
    ct = np.zeros((128, CW), np.float32)
    k = np.arange(128)
    ct[:, CO_CLS:CO_CLS + 32] = (k[:, None] % 32 == np.arange(32)[None, :])
    ct[:64, CO_BC32:CO_BC32 + 128] = (np.arange(128)[None, :] % 32 == k[:64, None] % 32)
    ct[:, CO_CCOL] = k // 32
    ct[:, CO_RG8:CO_RG8 + 16] = (k[:, None] // 8 == np.arange(16)[None, :])
    ct[:16, CO_RG4:CO_RG4 + 4] = (k[:16, None] // 4 == np.arange(4)[None, :])
    ct[:64, CO_ONES:CO_ONES + 64] = 1.0
    ct[:64, CO_QIOTA] = k[:64]
    ct[0, CO_PIOTA:CO_PIOTA + 64] = np.arange(64)
    ct[:64, CO_WMASK:CO_WMASK + 4] = (k[:64, None] // 16 == np.arange(4)[None, :])
    ct[:64, CO_WSEL:CO_WSEL + 16] = (k[:64, None] % 16 == np.arange(16)[None, :])
    ct[:4, CO_ID4:CO_ID4 + 4] = np.eye(4)
    ct[0, CO_ID1] = 1.0
    ct[:, CO_CGRP:CO_CGRP + 4] = (k[:, None] // 32 == np.arange(4)[None, :])
    ct[:4, CO_BLK:CO_BLK + 256] = (k[:4, None] == np.arange(256)[None, :] // 64)
    ct[:64, CO_WM4:CO_WM4 + 16] = (k[:64, None] // 16 == np.arange(16)[None, :] % 4)
    ct[:, CO_SMASK:CO_SMASK + 16] = (k[:, None] // 32 == np.arange(16)[None, :] // 4)
    ct[:16, CO_PISEL:CO_PISEL + 128] = (k[:16, None] == np.arange(128)[None, :] % 16)
    return ct


def _bcast_free(ap: bass.AP, dims) -> bass.AP:
    """Append/insert stride-0 free dims: dims is list of (pos, count) after
    existing free dims are kept; we just build [[part], *free..] manually."""
    new = [list(p) for p in ap.ap]
    for pos, count in dims:
        new.insert(pos, [0, count])
    return bass.AP(ap.tensor, ap.offset, new)


def build_core_kernel(debug: bool = False, dbg_out: bool = False) -> bacc.Bacc:
    nc = bacc.Bacc("TRN2", target_bir_lowering=False, debug=debug,
                   num_devices=N_CORES)

    pred_d = nc.dram_tensor("pred", [SPB, NCLS, IMG, IMG], F32, kind="ExternalInput")
    oimg_d = nc.dram_tensor("oimg", [SPB, 3, IMG, IMG], F32, kind="ExternalInput")
    aimg_d = nc.dram_tensor("aimg", [SPB, 3, IMG, IMG], F32, kind="ExternalInput")
    olab_d = nc.dram_tensor("olab", [SPB, IMG, IMG], I32, kind="ExternalInput")
    alab_d = nc.dram_tensor("alab", [SPB, IMG, IMG], I32, kind="ExternalInput")
    ocnf_d = nc.dram_tensor("ocnf", [SPB, IMG, IMG], F32, kind="ExternalInput")
    acnf_d = nc.dram_tensor("acnf", [SPB, IMG, IMG], F32, kind="ExternalInput")
    cst_d = nc.dram_tensor("consts", [128, CW], F32, kind="ExternalInput")

    if dbg_out:
        dbg_d = {
            "dbg_loss": nc.dram_tensor("dbg_loss", [NG, 1, 4], F32, kind="ExternalOutput"),
            "dbg_m": nc.dram_tensor("dbg_m", [NG, 1, 4], F32, kind="ExternalOutput"),
            "dbg_sg": nc.dram_tensor("dbg_sg", [NG, 1, 4], F32, kind="ExternalOutput"),
            "dbg_ss4": nc.dram_tensor("dbg_ss4", [NG, 4, 4], F32, kind="ExternalOutput"),
            "dbg_int4": nc.dram_tensor("dbg_int4", [NG, 4, 4], F32, kind="ExternalOutput"),
            "dbg_cnt4": nc.dram_tensor("dbg_cnt4", [NG, 4, 4], F32, kind="ExternalOutput"),
            "dbg_kT": nc.dram_tensor("dbg_kT", [NG, 4, 128], F32, kind="ExternalOutput"),
            "dbg_rp1": nc.dram_tensor("dbg_rp1", [NG, 1, 256], F32, kind="ExternalOutput"),
            "dbg_repl": nc.dram_tensor("dbg_repl", [NG, 1, 256], F32, kind="ExternalOutput"),
            "dbg_sidx": nc.dram_tensor("dbg_sidx", [NG, 1, 256], F32, kind="ExternalOutput"),
            "dbg_racs": nc.dram_tensor("dbg_racs", [NG, 64, 4], F32, kind="ExternalOutput"),
            "dbg_idxf": nc.dram_tensor("dbg_idxf", [NG, 128, 4], F32, kind="ExternalOutput"),
        }
    out_img = nc.dram_tensor("out_img", [SPB, 3, IMG, IMG], F32, kind="ExternalOutput")
    out_lab = nc.dram_tensor("out_lab", [SPB, IMG, IMG], I32, kind="ExternalOutput")
    out_cnf = nc.dram_tensor("out_cnf", [SPB, IMG, IMG], F32, kind="ExternalOutput")

    def patch_view(ap):
        # [ns, 256, 256] -> [ns, 32, 8, 256]  (r, bi, (bj w)) per sample
        return ap.rearrange("s (bi r) (bj w) -> s r bi (bj w)", bi=NH, bj=NH)

    with tile.TileContext(nc) as tc:
        with (
            tc.tile_pool(name="cstp", bufs=1) as cstp,
            tc.tile_pool(name="dice", bufs=2) as dice,
            tc.tile_pool(name="dice_e", bufs=GS) as dice_e,
            tc.tile_pool(name="cfinp", bufs=1) as cfinp,
            tc.tile_pool(name="parts", bufs=3 * NG) as parts,
            tc.tile_pool(name="keys", bufs=2) as keys,
            tc.tile_pool(name="tiny", bufs=8) as tiny,
            tc.tile_pool(name="chin", bufs=2) as chin,
            tc.tile_pool(name="chout", bufs=2) as chout,
            tc.tile_pool(name="ps512", bufs=4, space="PSUM") as ps512,
            tc.tile_pool(name="ps256", bufs=2, space="PSUM") as ps256,
            tc.tile_pool(name="pstiny", bufs=2, space="PSUM") as pstiny,
        ):
            ct = cstp.tile([128, CW], F32, tag="ct")
            nc.sync.dma_start(ct, cst_d[:])
            cls_ones = ct[:, CO_CLS:CO_CLS + 32]
            bc32 = ct[0:64, CO_BC32:CO_BC32 + 128]
            c_col = ct[:, CO_CCOL:CO_CCOL + 1]
            rg8 = ct[:, CO_RG8:CO_RG8 + 16]
            rg4 = ct[0:16, CO_RG4:CO_RG4 + 4]
            ones64 = ct[0:64, CO_ONES:CO_ONES + 64]
            qiota = ct[0:64, CO_QIOTA:CO_QIOTA + 1]
            piota = ct[0:1, CO_PIOTA:CO_PIOTA + 64]
            wsel = ct[0:64, CO_WSEL:CO_WSEL + 16]
            wmask4 = ct[0:64, CO_WM4:CO_WM4 + 16]
            smask = ct[:, CO_SMASK:CO_SMASK + 16]
            pisel = ct[0:16, CO_PISEL:CO_PISEL + 128]
            id4 = ct[0:4, CO_ID4:CO_ID4 + 4]
            id1 = ct[0:1, CO_ID1:CO_ID1 + 1]
            cls_grp = ct[:, CO_CGRP:CO_CGRP + 4]
            blkmask = ct[0:4, CO_BLK:CO_BLK + 256]

            for g in range(NG):
                s0 = g * GS
                ssl = slice(s0, s0 + GS)

                # ---------------- dice loss ----------------
                # pred tiles per sample: [128=(c,ph), 2048=pl]
                e_ts = []
                for si in range(GS):
                    pt = dice.tile([128, 2048], F32, tag="pred")
                    nc.sync.dma_start(
                        pt, pred_d[s0 + si].rearrange("c (a b) w -> (c a) (b w)", a=32))
                    et = dice_e.tile([128, 2048], F32, tag="e")
                    nc.scalar.activation(et, pt, AF.Exp)
                    e_ts.append(et)
                # labels as f32, replicated x4 on partitions via cast-DMA
                lbreps = []
                for si in range(GS):
                    lv = olab_d[s0 + si].rearrange("(a b) w -> a (b w)", a=32)
                    lrep = bass.AP(lv.tensor, lv.offset,
                                   [[0, 4]] + [list(p) for p in lv.ap])
                    lb = dice.tile([128, 2048], F32, tag="lbrep")
                    nc.gpsimd.dma_start(out=lb, in_=lrep)
                    lbreps.append(lb)

                scp = parts.tile([128, 4], F32, tag="part")
                intp = parts.tile([128, 4], F32, tag="part")
                cntp = parts.tile([128, 4], F32, tag="part")

                # class sums S -> r = exp(-ln(S)) per sample pair
                rps = []
                for hp in range(2):
                    rpair = dice.tile([64, 2048], F32, tag="rpair")
                    for ch in range(4):
                        cs = slice(ch * 512, ch * 512 + 512)
                        sps = ps512.tile([64, 512], F32, space="PSUM", tag="pbS")
                        for sj in range(2):
                            nc.tensor.matmul(sps[32 * sj:32 * sj + 32, :],
                                             lhsT=cls_ones,
                                             rhs=e_ts[2 * hp + sj][:, cs],
                                             start=True, stop=True)
                        lnt = dice.tile([64, 512], F32, tag="lnS")
                        nc.scalar.activation(lnt, sps, AF.Ln)
                        nc.scalar.activation(rpair[:, cs], lnt, AF.Exp, scale=-1.0)
                    rps.append(rpair)

                for si in range(GS):
                    hp, sj = si // 2, si % 2
                    col = slice(si, si + 1)
                    # r broadcast to (c, ph) partitions via sbuf->sbuf DMA
                    rb = dice.tile([128, 2048], F32, tag="rb")
                    for c in range(4):
                        nc.sync.dma_start(rb[32 * c:32 * c + 32, :],
                                          rps[hp][32 * sj:32 * sj + 32, :])
                    # s_c = e * r ; accum -> sum_s partial
                    scx = dice.tile([128, 2048], F32, tag="scx")
                    nc.vector.affine_mul_reduce(scx, scp[:, col],
                                                e_ts[si], rb, 1.0, 0.0)
                    # count: mask with accum; junk out reuses dead rb tile
                    nc.vector.tensor_scalar(rb, lbreps[si], c_col, None,
                                            AL.is_equal, AL.add,
                                            accum_out=cntp[:, col])
                    # inter = mask * s_c ; accum -> inter partial
                    nc.vector.scalar_tensor_tensor(
                        scx, lbreps[si], c_col, scx, AL.is_equal, AL.mult,
                        accum_out=intp[:, col])

                # reduce partials -> per (class, sample)
                q3 = []
                for ptile in (scp, intp, cntp):
                    pq = pstiny.tile([4, 4], F32, space="PSUM", tag="tp")
                    nc.tensor.matmul(pq, lhsT=cls_grp, rhs=ptile, start=True, stop=True)
                    q = tiny.tile([4, 4], F32, tag="q44")
                    nc.scalar.copy(q, pq)
                    q3.append(q)
                ss4, int4, cnt4 = q3
                u = tiny.tile([4, 4], F32, tag="q44")
                nc.vector.scalar_tensor_tensor(u, ss4, 1e-5, cnt4, AL.add, AL.add)
                iu = tiny.tile([4, 4], F32, tag="q44")
                nc.vector.reciprocal(iu, u)
                d4 = tiny.tile([4, 4], F32, tag="q44")
                nc.vector.scalar_tensor_tensor(d4, int4, 2.0, iu, AL.mult, AL.mult)
                lsump = pstiny.tile([1, 4], F32, space="PSUM", tag="tp")
                nc.tensor.matmul(lsump, lhsT=ones64[0:4, 0:1], rhs=d4,
                                 start=True, stop=True)
                loss = tiny.tile([1, 4], F32, tag="row4")
                nc.vector.tensor_scalar(loss, lsump, -1.0 / NCLS, 1.0,
                                        AL.mult, AL.add)
                trow = tiny.tile([1, 4], F32, tag="row4")
                nc.vector.tensor_scalar(trow, loss, -16.0 / AGE_DIV, 16.0,
                                        AL.mult, AL.add)
                ta = tiny.tile([1, 4], F32, tag="row4")
                nc.scalar.activation(ta, trow, AF.Abs)
                mrow = tiny.tile([1, 4], F32, tag="row4")
                nc.vector.tensor_scalar(mrow, ta, 16.0, None, AL.min)
                sg0 = tiny.tile([1, 4], F32, tag="row4")
                nc.vector.tensor_scalar(sg0, loss, 1.0, -2.0, AL.is_lt, AL.mult)
                sgrow = tiny.tile([1, 4], F32, tag="row4")
                nc.vector.tensor_scalar(sgrow, sg0, 1.0, None, AL.add)
                sgcp = pstiny.tile([4, 1], F32, space="PSUM", tag="tp")
                nc.tensor.transpose(sgcp, sgrow, id1)
                sgcol = tiny.tile([4, 1], F32, tag="col4")
                nc.scalar.copy(sgcol, sgcp)
                nsgcol = tiny.tile([4, 1], F32, tag="col4")
                nc.vector.tensor_scalar(nsgcol, sgcol, -1.0, None, AL.mult)

                # ---------------- keys / ranks / gather index ----------------
                cfin = cfinp.tile([128, 128 * 32], F32, tag="cfin")
                for si in range(GS):
                    row = slice(32 * si, 32 * si + 32)
                    nc.sync.dma_start(cfin[row, 0:2048],
                                      patch_view(ocnf_d[:])[s0 + si])
                    nc.sync.dma_start(cfin[row, 2048:4096],
                                      patch_view(acnf_d[:])[s0 + si])
                colred = keys.tile([128, 128], F32, tag="colred")
                nc.vector.tensor_reduce(colred,
                                        cfin[:].rearrange("p (n d) -> p n d", d=32),
                                        axis=mybir.AxisListType.X, op=AL.add)
                st1p = pstiny.tile([16, 128], F32, space="PSUM", tag="tp")
                nc.tensor.matmul(st1p, lhsT=rg8, rhs=colred, start=True, stop=True)
                st1 = keys.tile([16, 128], F32, tag="st1")
                nc.scalar.copy(st1, st1p)
                ksump = pstiny.tile([4, 128], F32, space="PSUM", tag="tp")
                nc.tensor.matmul(ksump, lhsT=rg4, rhs=st1, start=True, stop=True)
                kT = keys.tile([4, 128], F32, tag="kT")
                nc.vector.tensor_scalar(kT[:, 0:64], ksump[:, 0:64], sgcol[:, 0:1],
                                        None, AL.mult)
                nc.vector.tensor_scalar(kT[:, 64:128], ksump[:, 64:128],
                                        nsgcol[:, 0:1], None, AL.mult)
                kcp = pstiny.tile([128, 4], F32, space="PSUM", tag="tp")
                nc.tensor.transpose(kcp, kT, id4)
                kcols = keys.tile([128, 4], F32, tag="kcols")
                nc.scalar.copy(kcols, kcp)

                # block-diagonal expansion: rhs[k,(si,p)] = kT[k,p]*[si==k]
                blk3 = blkmask.rearrange("k (s p) -> k s p", s=4)
                kTeo = keys.tile([4, 256], F32, tag="kTe")
                nc.vector.tensor_tensor(
                    kTeo[:].rearrange("k (s p) -> k s p", s=4),
                    _bcast_free(kT[0:4, 0:64], [(1, 4)]), blk3, op=AL.mult)
                kTea = keys.tile([4, 256], F32, tag="kTe")
                nc.vector.tensor_tensor(
                    kTea[:].rearrange("k (s p) -> k s p", s=4),
                    _bcast_free(kT[0:4, 64:128], [(1, 4)]), blk3, op=AL.mult)
                obc = ps256.tile([64, 256], F32, space="PSUM", tag="k2")
                abc = ps256.tile([64, 256], F32, space="PSUM", tag="k2")
                nc.tensor.matmul(obc, lhsT=ones64[0:4, :], rhs=kTeo,
                                 start=True, stop=True)
                nc.tensor.matmul(abc, lhsT=ones64[0:4, :], rhs=kTea,
                                 start=True, stop=True)
                Ao = keys.tile([64, 256], F32, tag="Ao")
                nc.vector.tensor_tensor(
                    Ao, obc, _bcast_free(kcols[0:64, :], [(2, 64)]), op=AL.is_gt)
                Aa = keys.tile([64, 256], F32, tag="Aa")
                nc.vector.tensor_tensor(
                    Aa, abc, _bcast_free(kcols[64:128, :], [(2, 64)]), op=AL.is_gt)

                rorp = ps256.tile([64, 256], F32, space="PSUM", tag="k2")
                racp = pstiny.tile([64, 4], F32, space="PSUM", tag="tp")
                for si in range(GS):
                    osl = slice(64 * si, 64 * si + 64)
                    nc.tensor.matmul(rorp[:, osl], lhsT=ones64, rhs=Ao[:, osl],
                                     start=True, stop=True)
                    nc.tensor.matmul(racp[:, si:si + 1], lhsT=Aa[:, osl],
                                     rhs=ones64[:, 0:1], start=True, stop=True)
                racs = keys.tile([64, 4], F32, tag="racs")
                nc.scalar.copy(racs, racp)
                M2 = keys.tile([64, 256], F32, tag="M2")
                nc.vector.tensor_tensor(
                    M2, rorp, _bcast_free(racs[:, :], [(2, 64)]), op=AL.is_equal)
                srcp = pstiny.tile([1, 256], F32, space="PSUM", tag="tp")
                nc.tensor.matmul(srcp, lhsT=qiota, rhs=M2, start=True, stop=True)

                rp1 = keys.tile([1, 256], F32, tag="rowt")
                nc.vector.tensor_scalar(rp1, rorp[0:1, :], 1.0, None, AL.add)
                repl = keys.tile([1, 256], F32, tag="rowt")
                nc.vector.tensor_tensor(
                    repl, rp1, _bcast_free(mrow[0:1, :], [(2, 64)]), op=AL.is_le)
                x1 = keys.tile([1, 256], F32, tag="rowt")
                pio = _bcast_free(piota, [(1, 4)])
                nc.vector.tensor_tensor(x1, srcp, pio, op=AL.subtract)
                nc.vector.tensor_scalar(x1, x1, 64.0, None, AL.add)
                nc.vector.tensor_tensor(x1, x1, repl, op=AL.mult)
                nc.vector.tensor_tensor(x1, x1, pio, op=AL.add)

                sicp = pstiny.tile([64, 4], F32, space="PSUM", tag="tp")
                for si in range(GS):
                    nc.tensor.transpose(sicp[:, si:si + 1],
                                        x1[0:1, 64 * si:64 * si + 64], id1)
                sics = keys.tile([64, 4], F32, tag="sics")
                nc.scalar.copy(sics, sicp)
                # w1g[p, 4s+phi] = sidx_s[p] * [p//16 == phi]
                w1g = keys.tile([64, 16], F32, tag="w1g")
                nc.vector.tensor_tensor(
                    w1g[:].rearrange("p (s f) -> p s f", s=4),
                    _bcast_free(sics[:, :], [(2, 4)]),
                    wmask4.rearrange("p (s f) -> p s f", s=4), op=AL.mult)
                # wig[pi, 4s+phi] = sidx_s[16 phi + pi]
                wig = pstiny.tile([16, 16], F32, space="PSUM", tag="tp")
                nc.tensor.matmul(wig, lhsT=wsel, rhs=w1g, start=True, stop=True)
                wigs = keys.tile([16, 16], F32, tag="wigs")
                nc.scalar.copy(wigs, wig)
                # replicate rows to all 128 partitions
                wbig = ps256.tile([128, 16], F32, space="PSUM", tag="k2")
                nc.tensor.matmul(wbig, lhsT=pisel, rhs=wigs, start=True, stop=True)
                # select own sample block: mult by smask, reduce over s
                wsl = keys.tile([128, 16], F32, tag="wsl")
                nc.vector.tensor_tensor(wsl, wbig, smask, op=AL.mult)
                idxf = keys.tile([128, 4], F32, tag="idxf")
                ws_v = bass.AP(wsl[:].tensor, wsl[:].offset,
                               [list(wsl[:].ap[0]), [1, 4], [4, 4]])
                nc.vector.tensor_reduce(idxf, ws_v,
                                        axis=mybir.AxisListType.X, op=AL.add)
                idxt = keys.tile([128, 4], I16, tag="idxt")
                nc.vector.tensor_copy(idxt, idxf)

                if dbg_out:
                    for nm, t in [("dbg_loss", loss), ("dbg_m", mrow),
                                  ("dbg_sg", sgrow), ("dbg_ss4", ss4),
                                  ("dbg_int4", int4), ("dbg_cnt4", cnt4),
                                  ("dbg_kT", kT), ("dbg_rp1", rp1),
                                  ("dbg_repl", repl), ("dbg_sidx", x1),
                                  ("dbg_racs", racs), ("dbg_idxf", idxf)]:
                        nc.sync.dma_start(dbg_d[nm][g], t)

                # ---------------- gather + write back ----------------
                def store_chunk(cout, dstv):
                    for si in range(GS):
                        nc.sync.dma_start(
                            dstv[s0 + si],
                            cout[32 * si:32 * si + 32, :].rearrange(
                                "r (bi f) -> r bi f", bi=NH))

                def mix_chunk(osrcv, asrcv, dstv, dt):
                    cin = chin.tile([128, 128 * 32], dt, tag="cin")
                    for si in range(GS):
                        row = slice(32 * si, 32 * si + 32)
                        nc.sync.dma_start(cin[row, 0:2048], osrcv[s0 + si])
                        nc.sync.dma_start(cin[row, 2048:4096], asrcv[s0 + si])
                    cout = chout.tile([128, 64 * 32], dt, tag="cout")
                    nc.gpsimd.ap_gather(
                        cout[:].rearrange("p (n d) -> p n d", d=32),
                        cin[:].rearrange("p (n d) -> p n d", d=32),
                        idxt[:],
                        channels=128, num_elems=128, d=32, num_idxs=64)
                    store_chunk(cout, dstv)

                # conf chunk reuses cfin (already loaded)
                ccout = chout.tile([128, 64 * 32], F32, tag="cout")
                nc.gpsimd.ap_gather(
                    ccout[:].rearrange("p (n d) -> p n d", d=32),
                    cfin[:].rearrange("p (n d) -> p n d", d=32),
                    idxt[:],
                    channels=128, num_elems=128, d=32, num_idxs=64)
                store_chunk(ccout, patch_view(out_cnf[:]))
                for c in range(3):
                    mix_chunk(patch_view(oimg_d[:, c]), patch_view(aimg_d[:, c]),
                              patch_view(out_img[:, c]), F32)
                mix_chunk(patch_view(olab_d[:]), patch_view(alab_d[:]),
                          patch_view(out_lab[:]), I32)

    nc.compile()
    return nc


_cached = {}


def kernel(oimage, aimage, olabel, alabel, oconf, aconf, prediction,
           cur_step=None):
    if "nc" not in _cached:
        _cached["nc"] = build_core_kernel(debug=False)
    nc = _cached["nc"]
    cst = _build_consts()
    in_maps = []
    for i in range(N_CORES):
        sl = slice(i * SPB, (i + 1) * SPB)
        in_maps.append({
            "pred": np.ascontiguousarray(prediction[sl], np.float32),
            "oimg": np.ascontiguousarray(oimage[sl], np.float32),
            "aimg": np.ascontiguousarray(aimage[sl], np.float32),
            "olab": np.ascontiguousarray(olabel[sl], np.int32),
            "alab": np.ascontiguousarray(alabel[sl], np.int32),
            "ocnf": np.ascontiguousarray(oconf[sl], np.float32),
            "acnf": np.ascontiguousarray(aconf[sl], np.float32),
            "consts": cst,
        })
    res = run_bass_kernel_spmd(nc, in_maps, core_ids=list(range(N_CORES)))
    _exec_info["exec_time_ns"] = res.exec_time_ns
    img = np.concatenate([res.results[i]["out_img"] for i in range(N_CORES)])
    lab = np.concatenate([res.results[i]["out_lab"] for i in range(N_CORES)])
    cnf = np.concatenate([res.results[i]["out_cnf"] for i in range(N_CORES)])
    return img, lab.astype(np.int32), cnf


# revision 37
# speedup vs baseline: 1.4044x; 1.4044x over previous
"""AdaMix forward on 8 Trainium2 NeuronCores (Bass/Tile), pure data parallel.

Per core: 8 samples. Pipeline per 4-sample group:
  1. dice loss (classes-on-partitions layout; matmul class sums, fused
     AMR/STT accumulation) -> per-sample mix count k and sort sign.
  2. per-patch conf sums (segmented reduce + 2-stage matmuls), ranks via
     all-pairs compares, o<->a pairing via one-hot matmuls -> per-sample
     gather index over 128 patch slots (o patches 0..63, a 64..127).
  3. patch mix via gpsimd ap_gather per channel chunk; strided DMAs keep
     1KB contiguous runs on both sides.
"""

import os
import numpy as np

import concourse.bass as bass
import concourse.mybir as mybir
import concourse.tile as tile
from concourse import bacc
from concourse.bass_utils import run_bass_kernel_spmd

F32 = mybir.dt.float32
I32 = mybir.dt.int32
I16 = mybir.dt.int16
AL = mybir.AluOpType
AF = mybir.ActivationFunctionType

N_CORES = 8
B = 64
SPB = B // N_CORES      # samples per core
GS = 4                  # samples per group
NG = SPB // GS          # groups per core
NCLS = 4
IMG = 256
PS = 32                 # patch side
NH = 8                  # patches per image side
NP = 64                 # patches per image
AGE_DIV = 1.0 + 1e-5

# ---------------- constant pack (one [128, CW] f32 tensor) ----------------
CO_CLS = 0      # [128,32]  cls_ones:  k%32==m   (class sum over c)
CO_BC32 = 32    # [64,128]  bc32:      m%32==k%32 (broadcast ph -> (c,ph), x2 rows)
CO_CCOL = 160   # [128,1]   c_col:     k//32
CO_RG8 = 161    # [128,16]  rgrp8:     k//8==m
CO_RG4 = 177    # [16,4]    rgrp4:     k//4==m
CO_ONES = 181   # [64,64]   ones
CO_QIOTA = 245  # [64,1]    qiota:     k
CO_PIOTA = 246  # [1,64]    piota:     m
CO_WMASK = 310  # [64,4]    wrapmask:  k//16==m
CO_WSEL = 314   # [64,16]   wrapsel:   k%16==m
CO_ID4 = 330    # [4,4]     identity
CO_ID1 = 334    # [1,1]     identity
CO_CGRP = 335   # [128,4]   cls_grp:   k//32==m
CO_BLK = 339    # [4,256]   blkmask:   k==m//64
CO_WM4 = 595    # [64,16]   wmask4:    k//16==m%4
CO_SMASK = 611  # [128,16]  smask:     k//32==m//4
CO_PISEL = 627  # [16,128]  pisel:     k==m%16
CW = 756

_exec_info = {}


def _install_ntff_hook():
    """The agent image's antenv lacks axon_hooks; rebuild it from the boot
    helpers so trace=True (BASS_TRACE=1) works for profiling."""
    import sys
    import types
    try:
        import antenv.axon_hooks  # noqa: F401
        return
    except ImportError:
        pass
    try:
        import antenv
        from trn_agent_boot.trn_boot import _ntff_profile_via_ctypes
        hook = _ntff_profile_via_ctypes("/opt/axon/libaxon_pjrt.so")
        mod = types.ModuleType("antenv.axon_hooks")
        state = {"hook": hook}
        mod.set_axon_ntff_profile_hook = lambda h: state.update(hook=h)
        mod.get_axon_ntff_profile_hook = lambda: state["hook"]
        sys.modules["antenv.axon_hooks"] = mod
        antenv.axon_hooks = mod
    except Exception:
        pass


_install_ntff_hook()


def _build_consts() -> np.ndarray:
    ct = np.zeros((128, CW), np.float32)
    k = np.arange(128)
    ct[:, CO_CLS:CO_CLS + 32] = (k[:, None] % 32 == np.arange(32)[None, :])
    ct[:64, CO_BC32:CO_BC32 + 128] = (np.arange(128)[None, :] % 32 == k[:64, None] % 32)
    ct[:, CO_CCOL] = k // 32
    ct[:, CO_RG8:CO_RG8 + 16] = (k[:, None] // 8 == np.arange(16)[None, :])
    ct[:16, CO_RG4:CO_RG4 + 4] = (k[:16, None] // 4 == np.arange(4)[None, :])
    ct[:64, CO_ONES:CO_ONES + 64] = 1.0
    ct[:64, CO_QIOTA] = k[:64]
    ct[0, CO_PIOTA:CO_PIOTA + 64] = np.arange(64)
    ct[:64, CO_WMASK:CO_WMASK + 4] = (k[:64, None] // 16 == np.arange(4)[None, :])
    ct[:64, CO_WSEL:CO_WSEL + 16] = (k[:64, None] % 16 == np.arange(16)[None, :])
    ct[:4, CO_ID4:CO_ID4 + 4] = np.eye(4)
    ct[0, CO_ID1] = 1.0
    ct[:, CO_CGRP:CO_CGRP + 4] = (k[:, None] // 32 == np.arange(4)[None, :])
    ct[:4, CO_BLK:CO_BLK + 256] = (k[:4, None] == np.arange(256)[None, :] // 64)
    ct[:64, CO_WM4:CO_WM4 + 16] = (k[:64, None] // 16 == np.arange(16)[None, :] % 4)
    ct[:, CO_SMASK:CO_SMASK + 16] = (k[:, None] // 32 == np.arange(16)[None, :] // 4)
    ct[:16, CO_PISEL:CO_PISEL + 128] = (k[:16, None] == np.arange(128)[None, :] % 16)
    return ct


def _bcast_free(ap: bass.AP, dims) -> bass.AP:
    """Append/insert stride-0 free dims: dims is list of (pos, count) after
    existing free dims are kept; we just build [[part], *free..] manually."""
    new = [list(p) for p in ap.ap]
    for pos, count in dims:
        new.insert(pos, [0, count])
    return bass.AP(ap.tensor, ap.offset, new)


def build_core_kernel(debug: bool = False, dbg_out: bool = False) -> bacc.Bacc:
    nc = bacc.Bacc("TRN2", target_bir_lowering=False, debug=debug,
                   num_devices=N_CORES)

    pred_d = nc.dram_tensor("pred", [SPB, NCLS, IMG, IMG], F32, kind="ExternalInput")
    oimg_d = nc.dram_tensor("oimg", [SPB, 3, IMG, IMG], F32, kind="ExternalInput")
    aimg_d = nc.dram_tensor("aimg", [SPB, 3, IMG, IMG], F32, kind="ExternalInput")
    olab_d = nc.dram_tensor("olab", [SPB, IMG, IMG], I32, kind="ExternalInput")
    alab_d = nc.dram_tensor("alab", [SPB, IMG, IMG], I32, kind="ExternalInput")
    ocnf_d = nc.dram_tensor("ocnf", [SPB, IMG, IMG], F32, kind="ExternalInput")
    acnf_d = nc.dram_tensor("acnf", [SPB, IMG, IMG], F32, kind="ExternalInput")
    cst_d = nc.dram_tensor("consts", [128, CW], F32, kind="ExternalInput")

    if dbg_out:
        dbg_d = {
            "dbg_loss": nc.dram_tensor("dbg_loss", [NG, 1, 4], F32, kind="ExternalOutput"),
            "dbg_m": nc.dram_tensor("dbg_m", [NG, 1, 4], F32, kind="ExternalOutput"),
            "dbg_sg": nc.dram_tensor("dbg_sg", [NG, 1, 4], F32, kind="ExternalOutput"),
            "dbg_ss4": nc.dram_tensor("dbg_ss4", [NG, 4, 4], F32, kind="ExternalOutput"),
            "dbg_int4": nc.dram_tensor("dbg_int4", [NG, 4, 4], F32, kind="ExternalOutput"),
            "dbg_cnt4": nc.dram_tensor("dbg_cnt4", [NG, 4, 4], F32, kind="ExternalOutput"),
            "dbg_kT": nc.dram_tensor("dbg_kT", [NG, 4, 128], F32, kind="ExternalOutput"),
            "dbg_rp1": nc.dram_tensor("dbg_rp1", [NG, 1, 256], F32, kind="ExternalOutput"),
            "dbg_repl": nc.dram_tensor("dbg_repl", [NG, 1, 256], F32, kind="ExternalOutput"),
            "dbg_sidx": nc.dram_tensor("dbg_sidx", [NG, 1, 256], F32, kind="ExternalOutput"),
            "dbg_racs": nc.dram_tensor("dbg_racs", [NG, 64, 4], F32, kind="ExternalOutput"),
            "dbg_idxf": nc.dram_tensor("dbg_idxf", [NG, 128, 4], F32, kind="ExternalOutput"),
        }
    out_img = nc.dram_tensor("out_img", [SPB, 3, IMG, IMG], F32, kind="ExternalOutput")
    out_lab = nc.dram_tensor("out_lab", [SPB, IMG, IMG], I32, kind="ExternalOutput")
    out_cnf = nc.dram_tensor("out_cnf", [SPB, IMG, IMG], F32, kind="ExternalOutput")

    def patch_view(ap):
        # [ns, 256, 256] -> [ns, 32, 8, 256]  (r, bi, (bj w)) per sample
        return ap.rearrange("s (bi r) (bj w) -> s r bi (bj w)", bi=NH, bj=NH)

    with tile.TileContext(nc) as tc:
        with (
            tc.tile_pool(name="cstp", bufs=1) as cstp,
            tc.tile_pool(name="predp", bufs=1) as predp,
            tc.tile_pool(name="dice", bufs=2) as dice,
            tc.tile_pool(name="dice_e", bufs=3) as dice_e,
            tc.tile_pool(name="parts", bufs=3 * NG) as parts,
            tc.tile_pool(name="keys", bufs=2) as keys,
            tc.tile_pool(name="tiny", bufs=8) as tiny,
            tc.tile_pool(name="chin", bufs=4) as chin,
            tc.tile_pool(name="chout", bufs=3) as chout,
            tc.tile_pool(name="ps512", bufs=3, space="PSUM") as ps512,
            tc.tile_pool(name="psS", bufs=2, space="PSUM") as psS,
            tc.tile_pool(name="ps256", bufs=2, space="PSUM") as ps256,
            tc.tile_pool(name="pstiny", bufs=1, space="PSUM") as pstiny,
        ):
            ct = cstp.tile([128, CW], F32, tag="ct")
            nc.sync.dma_start(ct, cst_d[:])
            cls_ones = ct[:, CO_CLS:CO_CLS + 32]
            bc32 = ct[0:64, CO_BC32:CO_BC32 + 128]
            c_col = ct[:, CO_CCOL:CO_CCOL + 1]
            rg8 = ct[:, CO_RG8:CO_RG8 + 16]
            rg4 = ct[0:16, CO_RG4:CO_RG4 + 4]
            ones64 = ct[0:64, CO_ONES:CO_ONES + 64]
            qiota = ct[0:64, CO_QIOTA:CO_QIOTA + 1]
            piota = ct[0:1, CO_PIOTA:CO_PIOTA + 64]
            wsel = ct[0:64, CO_WSEL:CO_WSEL + 16]
            wmask4 = ct[0:64, CO_WM4:CO_WM4 + 16]
            smask = ct[:, CO_SMASK:CO_SMASK + 16]
            pisel = ct[0:16, CO_PISEL:CO_PISEL + 128]
            id4 = ct[0:4, CO_ID4:CO_ID4 + 4]
            id1 = ct[0:1, CO_ID1:CO_ID1 + 1]
            cls_grp = ct[:, CO_CGRP:CO_CGRP + 4]
            blkmask = ct[0:4, CO_BLK:CO_BLK + 256]
            bc32 = ct[0:64, CO_BC32:CO_BC32 + 128]
            # bf16 copy of bc32 for the label-broadcast matmuls
            bc32h = cstp.tile([64, 128], mybir.dt.bfloat16, tag="bc32h")
            nc.vector.tensor_copy(bc32h, bc32)

            for g in range(NG):
                s0 = g * GS
                ssl = slice(s0, s0 + GS)

                # ---------------- dice loss ----------------
                scp = parts.tile([128, 16], F32, tag="part")
                intp = parts.tile([128, 16], F32, tag="part")
                cntp = parts.tile([128, 16], F32, tag="part")
                labv = olab_d[:].rearrange("s (a b) w -> s a (b w)", a=32)

                for hp in range(2):
                    # pred tiles per sample: [128=(c,ph), 2048=pl]
                    e_ts = []
                    for sj in range(2):
                        si = 2 * hp + sj
                        pt = predp.tile([128, 2048], F32, tag="pred")
                        nc.sync.dma_start(
                            pt, pred_d[s0 + si].rearrange(
                                "c (a b) w -> (c a) (b w)", a=32))
                        et = dice_e.tile([128, 2048], F32, tag="e")
                        nc.scalar.activation(et, pt, AF.Exp)
                        e_ts.append(et)
                    # labels pair tile as bf16 [64=(s2,ph), 2048] (exact 0..3)
                    lf = dice.tile([64, 2048], mybir.dt.bfloat16, tag="labf")
                    nc.gpsimd.dma_start(
                        out=lf, in_=labv[s0 + 2 * hp:s0 + 2 * hp + 2])
                    # class sums S -> r = exp(-ln(S))
                    rpair = dice.tile([64, 2048], F32, tag="rpair")
                    for ch in range(4):
                        cs = slice(ch * 512, ch * 512 + 512)
                        sps = psS.tile([64, 512], F32, space="PSUM", tag="pbS")
                        for sj in range(2):
                            nc.tensor.matmul(sps[32 * sj:32 * sj + 32, :],
                                             lhsT=cls_ones,
                                             rhs=e_ts[sj][:, cs],
                                             start=True, stop=True)
                        lnt = dice.tile([64, 512], F32, tag="lnS")
                        nc.scalar.activation(lnt, sps, AF.Ln)
                        nc.scalar.activation(rpair[:, cs], lnt, AF.Exp, scale=-1.0)

                    for sj in range(2):
                        si = 2 * hp + sj
                        psl = slice(32 * sj, 32 * sj + 32)
                        for ch in range(4):
                            cs = slice(ch * 512, ch * 512 + 512)
                            col = slice(4 * si + ch, 4 * si + ch + 1)
                            lbp = ps512.tile([128, 512], F32, space="PSUM", tag="pb")
                            nc.tensor.matmul(lbp, lhsT=bc32h[psl, :],
                                             rhs=lf[psl, cs], start=True, stop=True)
                            rbp = ps512.tile([128, 512], F32, space="PSUM", tag="pb")
                            nc.tensor.matmul(rbp, lhsT=bc32[psl, :],
                                             rhs=rpair[psl, cs], start=True, stop=True)
                            # count: mask with accum (junk out tile)
                            mjk = dice.tile([128, 512], F32, tag="scx")
                            nc.any.tensor_scalar(mjk, lbp, c_col, None,
                                                 AL.is_equal, AL.add,
                                                 accum_out=cntp[:, col])
                            # s_c = e * r ; accum -> sum_s partial
                            scx = dice.tile([128, 512], F32, tag="scx")
                            nc.vector.affine_mul_reduce(
                                scx, scp[:, col], e_ts[sj][:, cs], rbp, 1.0, 0.0)
                            # inter = mask * s_c ; accum -> inter partial
                            nc.vector.scalar_tensor_tensor(
                                scx, lbp, c_col, scx, AL.is_equal, AL.mult,
                                accum_out=intp[:, col])

                # reduce partials -> per (class, sample)
                q3 = []
                for ptile in (scp, intp, cntp):
                    pq = pstiny.tile([4, 16], F32, space="PSUM", tag="tp")
                    nc.tensor.matmul(pq, lhsT=cls_grp, rhs=ptile, start=True, stop=True)
                    q = tiny.tile([4, 4], F32, tag="q44")
                    nc.vector.tensor_reduce(q, pq.rearrange("c (s h) -> c s h", s=4),
                                            axis=mybir.AxisListType.X, op=AL.add)
                    q3.append(q)
                ss4, int4, cnt4 = q3
                u = tiny.tile([4, 4], F32, tag="q44")
                nc.vector.scalar_tensor_tensor(u, ss4, 1e-5, cnt4, AL.add, AL.add)
                iu = tiny.tile([4, 4], F32, tag="q44")
                nc.vector.reciprocal(iu, u)
                d4 = tiny.tile([4, 4], F32, tag="q44")
                nc.vector.scalar_tensor_tensor(d4, int4, 2.0, iu, AL.mult, AL.mult)
                lsump = pstiny.tile([1, 4], F32, space="PSUM", tag="tp")
                nc.tensor.matmul(lsump, lhsT=ones64[0:4, 0:1], rhs=d4,
                                 start=True, stop=True)
                loss = tiny.tile([1, 4], F32, tag="row4")
                nc.vector.tensor_scalar(loss, lsump, -1.0 / NCLS, 1.0,
                                        AL.mult, AL.add)
                trow = tiny.tile([1, 4], F32, tag="row4")
                nc.vector.tensor_scalar(trow, loss, -16.0 / AGE_DIV, 16.0,
                                        AL.mult, AL.add)
                ta = tiny.tile([1, 4], F32, tag="row4")
                nc.scalar.activation(ta, trow, AF.Abs)
                mrow = tiny.tile([1, 4], F32, tag="row4")
                nc.vector.tensor_scalar(mrow, ta, 16.0, None, AL.min)
                sg0 = tiny.tile([1, 4], F32, tag="row4")
                nc.vector.tensor_scalar(sg0, loss, 1.0, -2.0, AL.is_lt, AL.mult)
                sgrow = tiny.tile([1, 4], F32, tag="row4")
                nc.vector.tensor_scalar(sgrow, sg0, 1.0, None, AL.add)
                sgcp = pstiny.tile([4, 1], F32, space="PSUM", tag="tp")
                nc.tensor.transpose(sgcp, sgrow, id1)
                sgcol = tiny.tile([4, 1], F32, tag="col4")
                nc.scalar.copy(sgcol, sgcp)
                nsgcol = tiny.tile([4, 1], F32, tag="col4")
                nc.vector.tensor_scalar(nsgcol, sgcol, -1.0, None, AL.mult)

                # ---------------- keys / ranks / gather index ----------------
                cfin = chin.tile([128, 128 * 32], F32, tag="cin")
                for si in range(GS):
                    row = slice(32 * si, 32 * si + 32)
                    nc.sync.dma_start(cfin[row, 0:2048],
                                      patch_view(ocnf_d[:])[s0 + si])
                    nc.sync.dma_start(cfin[row, 2048:4096],
                                      patch_view(acnf_d[:])[s0 + si])
                colred = keys.tile([128, 128], F32, tag="colred")
                nc.vector.tensor_reduce(colred,
                                        cfin[:].rearrange("p (n d) -> p n d", d=32),
                                        axis=mybir.AxisListType.X, op=AL.add)
                st1p = pstiny.tile([16, 128], F32, space="PSUM", tag="tp")
                nc.tensor.matmul(st1p, lhsT=rg8, rhs=colred, start=True, stop=True)
                st1 = keys.tile([16, 128], F32, tag="st1")
                nc.scalar.copy(st1, st1p)
                ksump = pstiny.tile([4, 128], F32, space="PSUM", tag="tp")
                nc.tensor.matmul(ksump, lhsT=rg4, rhs=st1, start=True, stop=True)
                kT = keys.tile([4, 128], F32, tag="kT")
                nc.vector.tensor_scalar(kT[:, 0:64], ksump[:, 0:64], sgcol[:, 0:1],
                                        None, AL.mult)
                nc.vector.tensor_scalar(kT[:, 64:128], ksump[:, 64:128],
                                        nsgcol[:, 0:1], None, AL.mult)
                kcp = pstiny.tile([128, 4], F32, space="PSUM", tag="tp")
                nc.tensor.transpose(kcp, kT, id4)
                kcols = keys.tile([128, 4], F32, tag="kcols")
                nc.scalar.copy(kcols, kcp)

                # block-diagonal expansion: rhs[k,(si,p)] = kT[k,p]*[si==k]
                blk3 = blkmask.rearrange("k (s p) -> k s p", s=4)
                kTeo = keys.tile([4, 256], F32, tag="kTe")
                nc.vector.tensor_tensor(
                    kTeo[:].rearrange("k (s p) -> k s p", s=4),
                    _bcast_free(kT[0:4, 0:64], [(1, 4)]), blk3, op=AL.mult)
                kTea = keys.tile([4, 256], F32, tag="kTe")
                nc.vector.tensor_tensor(
                    kTea[:].rearrange("k (s p) -> k s p", s=4),
                    _bcast_free(kT[0:4, 64:128], [(1, 4)]), blk3, op=AL.mult)
                obc = ps256.tile([64, 256], F32, space="PSUM", tag="k2")
                abc = ps256.tile([64, 256], F32, space="PSUM", tag="k2")
                nc.tensor.matmul(obc, lhsT=ones64[0:4, :], rhs=kTeo,
                                 start=True, stop=True)
                nc.tensor.matmul(abc, lhsT=ones64[0:4, :], rhs=kTea,
                                 start=True, stop=True)
                Ao = keys.tile([64, 256], F32, tag="Ao")
                nc.vector.tensor_tensor(
                    Ao, obc, _bcast_free(kcols[0:64, :], [(2, 64)]), op=AL.is_gt)
                Aa = keys.tile([64, 256], F32, tag="Aa")
                nc.vector.tensor_tensor(
                    Aa, abc, _bcast_free(kcols[64:128, :], [(2, 64)]), op=AL.is_gt)

                rorp = ps256.tile([64, 256], F32, space="PSUM", tag="k2")
                racp = pstiny.tile([64, 4], F32, space="PSUM", tag="tp")
                for si in range(GS):
                    osl = slice(64 * si, 64 * si + 64)
                    nc.tensor.matmul(rorp[:, osl], lhsT=ones64, rhs=Ao[:, osl],
                                     start=True, stop=True)
                    nc.tensor.matmul(racp[:, si:si + 1], lhsT=Aa[:, osl],
                                     rhs=ones64[:, 0:1], start=True, stop=True)
                racs = keys.tile([64, 4], F32, tag="racs")
                nc.scalar.copy(racs, racp)
                M2 = keys.tile([64, 256], F32, tag="M2")
                nc.vector.tensor_tensor(
                    M2, rorp, _bcast_free(racs[:, :], [(2, 64)]), op=AL.is_equal)
                srcp = pstiny.tile([1, 256], F32, space="PSUM", tag="tp")
                nc.tensor.matmul(srcp, lhsT=qiota, rhs=M2, start=True, stop=True)

                rp1 = keys.tile([1, 256], F32, tag="rowt")
                nc.vector.tensor_scalar(rp1, rorp[0:1, :], 1.0, None, AL.add)
                repl = keys.tile([1, 256], F32, tag="rowt")
                nc.vector.tensor_tensor(
                    repl, rp1, _bcast_free(mrow[0:1, :], [(2, 64)]), op=AL.is_le)
                x1 = keys.tile([1, 256], F32, tag="rowt")
                pio = _bcast_free(piota, [(1, 4)])
                nc.vector.tensor_tensor(x1, srcp, pio, op=AL.subtract)
                nc.vector.tensor_scalar(x1, x1, 64.0, None, AL.add)
                nc.vector.tensor_tensor(x1, x1, repl, op=AL.mult)
                nc.vector.tensor_tensor(x1, x1, pio, op=AL.add)

                sicp = pstiny.tile([64, 4], F32, space="PSUM", tag="tp")
                for si in range(GS):
                    nc.tensor.transpose(sicp[:, si:si + 1],
                                        x1[0:1, 64 * si:64 * si + 64], id1)
                sics = keys.tile([64, 4], F32, tag="sics")
                nc.scalar.copy(sics, sicp)
                # w1g[p, 4s+phi] = sidx_s[p] * [p//16 == phi]
                w1g = keys.tile([64, 16], F32, tag="w1g")
                nc.vector.tensor_tensor(
                    w1g[:].rearrange("p (s f) -> p s f", s=4),
                    _bcast_free(sics[:, :], [(2, 4)]),
                    wmask4.rearrange("p (s f) -> p s f", s=4), op=AL.mult)
                # wig[pi, 4s+phi] = sidx_s[16 phi + pi]
                wig = pstiny.tile([16, 16], F32, space="PSUM", tag="tp")
                nc.tensor.matmul(wig, lhsT=wsel, rhs=w1g, start=True, stop=True)
                wigs = keys.tile([16, 16], F32, tag="wigs")
                nc.scalar.copy(wigs, wig)
                # replicate rows to all 128 partitions
                wbig = ps256.tile([128, 16], F32, space="PSUM", tag="k2")
                nc.tensor.matmul(wbig, lhsT=pisel, rhs=wigs, start=True, stop=True)
                # select own sample block: mult by smask, reduce over s
                wsl = keys.tile([128, 16], F32, tag="wsl")
                nc.vector.tensor_tensor(wsl, wbig, smask, op=AL.mult)
                idxf = keys.tile([128, 4], F32, tag="idxf")
                ws_v = bass.AP(wsl[:].tensor, wsl[:].offset,
                               [list(wsl[:].ap[0]), [1, 4], [4, 4]])
                nc.vector.tensor_reduce(idxf, ws_v,
                                        axis=mybir.AxisListType.X, op=AL.add)
                idxt = keys.tile([128, 4], I16, tag="idxt")
                nc.vector.tensor_copy(idxt, idxf)

                if dbg_out:
                    for nm, t in [("dbg_loss", loss), ("dbg_m", mrow),
                                  ("dbg_sg", sgrow), ("dbg_ss4", ss4),
                                  ("dbg_int4", int4), ("dbg_cnt4", cnt4),
                                  ("dbg_kT", kT), ("dbg_rp1", rp1),
                                  ("dbg_repl", repl), ("dbg_sidx", x1),
                                  ("dbg_racs", racs), ("dbg_idxf", idxf)]:
                        nc.sync.dma_start(dbg_d[nm][g], t)

                # ---------------- gather + write back ----------------
                def store_chunk(cout, dstv):
                    for si in range(GS):
                        nc.sync.dma_start(
                            dstv[s0 + si],
                            cout[32 * si:32 * si + 32, :].rearrange(
                                "r (bi f) -> r bi f", bi=NH))

                def mix_chunk(osrcv, asrcv, dstv, dt):
                    cin = chin.tile([128, 128 * 32], dt, tag="cin")
                    for si in range(GS):
                        row = slice(32 * si, 32 * si + 32)
                        nc.sync.dma_start(cin[row, 0:2048], osrcv[s0 + si])
                        nc.sync.dma_start(cin[row, 2048:4096], asrcv[s0 + si])
                    cout = chout.tile([128, 64 * 32], dt, tag="cout")
                    nc.gpsimd.ap_gather(
                        cout[:].rearrange("p (n d) -> p n d", d=32),
                        cin[:].rearrange("p (n d) -> p n d", d=32),
                        idxt[:],
                        channels=128, num_elems=128, d=32, num_idxs=64)
                    store_chunk(cout, dstv)

                # conf chunk reuses cfin (already loaded)
                ccout = chout.tile([128, 64 * 32], F32, tag="cout")
                nc.gpsimd.ap_gather(
                    ccout[:].rearrange("p (n d) -> p n d", d=32),
                    cfin[:].rearrange("p (n d) -> p n d", d=32),
                    idxt[:],
                    channels=128, num_elems=128, d=32, num_idxs=64)
                store_chunk(ccout, patch_view(out_cnf[:]))
                for c in range(3):
                    mix_chunk(patch_view(oimg_d[:, c]), patch_view(aimg_d[:, c]),
                              patch_view(out_img[:, c]), F32)
                mix_chunk(patch_view(olab_d[:]), patch_view(alab_d[:]),
                          patch_view(out_lab[:]), I32)

    nc.compile()
    return nc


_cached = {}


def kernel(oimage, aimage, olabel, alabel, oconf, aconf, prediction,
           cur_step=None):
    if "nc" not in _cached:
        _cached["nc"] = build_core_kernel(debug=False)
    nc = _cached["nc"]
    cst = _build_consts()
    in_maps = []
    for i in range(N_CORES):
        sl = slice(i * SPB, (i + 1) * SPB)
        in_maps.append({
            "pred": np.ascontiguousarray(prediction[sl], np.float32),
            "oimg": np.ascontiguousarray(oimage[sl], np.float32),
            "aimg": np.ascontiguousarray(aimage[sl], np.float32),
            "olab": np.ascontiguousarray(olabel[sl], np.int32),
            "alab": np.ascontiguousarray(alabel[sl], np.int32),
            "ocnf": np.ascontiguousarray(oconf[sl], np.float32),
            "acnf": np.ascontiguousarray(aconf[sl], np.float32),
            "consts": cst,
        })
    res = run_bass_kernel_spmd(nc, in_maps, core_ids=list(range(N_CORES)))
    _exec_info["exec_time_ns"] = res.exec_time_ns
    img = np.concatenate([res.results[i]["out_img"] for i in range(N_CORES)])
    lab = np.concatenate([res.results[i]["out_lab"] for i in range(N_CORES)])
    cnf = np.concatenate([res.results[i]["out_cnf"] for i in range(N_CORES)])
    return img, lab.astype(np.int32), cnf


# revision 39
# speedup vs baseline: 1.5496x; 1.1034x over previous
"""AdaMix forward on 8 Trainium2 NeuronCores (Bass/Tile), pure data parallel.

Per core: 8 samples, processed as 2 groups of 4. Phases (heavily pipelined by
Tile): per group -> dice loss (classes-on-partitions, replicated class-sum
matmul, r = exp(-ln(S)) on ScalarE, fused AMR/STT accumulation), conf patch
sums + sign-independent ranks (all-pairs compares + one-hot matmuls), tiny
sign/k fixup -> per-sample gather index; then all channel chunks stream
through gpsimd ap_gather (o patches = slots 0..63, a = 64..127).
"""

import os
import numpy as np

import concourse.bass as bass
import concourse.mybir as mybir
import concourse.tile as tile
from concourse import bacc
from concourse.bass_utils import run_bass_kernel_spmd

F32 = mybir.dt.float32
BF16 = mybir.dt.bfloat16
I32 = mybir.dt.int32
I16 = mybir.dt.int16
AL = mybir.AluOpType
AF = mybir.ActivationFunctionType

N_CORES = 8
B = 64
SPB = B // N_CORES      # samples per core
GS = 4                  # samples per group
NG = SPB // GS          # groups per core
NCLS = 4
IMG = 256
PS = 32                 # patch side
NH = 8                  # patches per image side
NP = 64                 # patches per image
AGE_DIV = 1.0 + 1e-5

# ---------------- constant pack (one [128, CW] f32 tensor) ----------------
CO_CLS = 0      # [128,32]  cls_ones:  k%32==m   (class sum over c)
CO_BC32 = 32    # [128,128] bc32:      m%32==k%32 (replicated broadcast)
CO_CCOL = 160   # [128,1]   c_col:     k//32
CO_RG8 = 161    # [128,16]  rgrp8:     k//8==m
CO_RG4 = 177    # [16,4]    rgrp4:     k//4==m
CO_ONES = 181   # [64,64]   ones
CO_QIOTA = 245  # [64,1]    qiota:     k
CO_PIOTA = 246  # [1,64]    piota:     m
CO_WMASK = 310  # [64,4]    (unused)
CO_WSEL = 314   # [64,16]   wrapsel:   k%16==m
CO_ID4 = 330    # [4,4]     identity
CO_ID1 = 334    # [1,1]     identity
CO_CGRP = 335   # [128,4]   cls_grp:   k//32==m
CO_BLK = 339    # [4,256]   blkmask:   k==m//64
CO_WM4 = 595    # [64,16]   wmask4:    k//16==m%4
CO_SMASK = 611  # [128,16]  smask:     k//32==m//4
CO_PISEL = 627  # [16,128]  pisel:     k==m%16
CW = 756

_exec_info = {}


def _install_ntff_hook():
    """The agent image's antenv lacks axon_hooks; rebuild it from the boot
    helpers so trace=True (BASS_TRACE=1) works for profiling."""
    import sys
    import types
    try:
        import antenv.axon_hooks  # noqa: F401
        return
    except ImportError:
        pass
    try:
        import antenv
        from trn_agent_boot.trn_boot import _ntff_profile_via_ctypes
        hook = _ntff_profile_via_ctypes("/opt/axon/libaxon_pjrt.so")
        mod = types.ModuleType("antenv.axon_hooks")
        state = {"hook": hook}
        mod.set_axon_ntff_profile_hook = lambda h: state.update(hook=h)
        mod.get_axon_ntff_profile_hook = lambda: state["hook"]
        sys.modules["antenv.axon_hooks"] = mod
        antenv.axon_hooks = mod
    except Exception:
        pass


_install_ntff_hook()


def _build_consts() -> np.ndarray:
    ct = np.zeros((128, CW), np.float32)
    k = np.arange(128)
    ct[:, CO_CLS:CO_CLS + 32] = (k[:, None] % 32 == np.arange(32)[None, :])
    ct[:, CO_BC32:CO_BC32 + 128] = (np.arange(128)[None, :] % 32 == k[:, None] % 32)
    ct[:, CO_CCOL] = k // 32
    ct[:, CO_RG8:CO_RG8 + 16] = (k[:, None] // 8 == np.arange(16)[None, :])
    ct[:16, CO_RG4:CO_RG4 + 4] = (k[:16, None] // 4 == np.arange(4)[None, :])
    ct[:64, CO_ONES:CO_ONES + 64] = 1.0
    ct[:64, CO_QIOTA] = k[:64]
    ct[0, CO_PIOTA:CO_PIOTA + 64] = np.arange(64)
    ct[:64, CO_WSEL:CO_WSEL + 16] = (k[:64, None] % 16 == np.arange(16)[None, :])
    ct[:4, CO_ID4:CO_ID4 + 4] = np.eye(4)
    ct[0, CO_ID1] = 1.0
    ct[:, CO_CGRP:CO_CGRP + 4] = (k[:, None] // 32 == np.arange(4)[None, :])
    ct[:4, CO_BLK:CO_BLK + 256] = (k[:4, None] == np.arange(256)[None, :] // 64)
    ct[:64, CO_WM4:CO_WM4 + 16] = (k[:64, None] // 16 == np.arange(16)[None, :] % 4)
    ct[:, CO_SMASK:CO_SMASK + 16] = (k[:, None] // 32 == np.arange(16)[None, :] // 4)
    ct[:16, CO_PISEL:CO_PISEL + 128] = (k[:16, None] == np.arange(128)[None, :] % 16)
    return ct


def _bcast_free(ap: bass.AP, dims) -> bass.AP:
    """Insert stride-0 free dims at the given (pos, count) positions."""
    new = [list(p) for p in ap.ap]
    for pos, count in dims:
        new.insert(pos, [0, count])
    return bass.AP(ap.tensor, ap.offset, new)


def build_core_kernel(debug: bool = False, dbg_out: bool = False) -> bacc.Bacc:
    nc = bacc.Bacc("TRN2", target_bir_lowering=False, debug=debug,
                   num_devices=N_CORES)

    pred_d = nc.dram_tensor("pred", [SPB, NCLS, IMG, IMG], F32, kind="ExternalInput")
    oimg_d = nc.dram_tensor("oimg", [SPB, 3, IMG, IMG], F32, kind="ExternalInput")
    aimg_d = nc.dram_tensor("aimg", [SPB, 3, IMG, IMG], F32, kind="ExternalInput")
    olab_d = nc.dram_tensor("olab", [SPB, IMG, IMG], I32, kind="ExternalInput")
    alab_d = nc.dram_tensor("alab", [SPB, IMG, IMG], I32, kind="ExternalInput")
    ocnf_d = nc.dram_tensor("ocnf", [SPB, IMG, IMG], F32, kind="ExternalInput")
    acnf_d = nc.dram_tensor("acnf", [SPB, IMG, IMG], F32, kind="ExternalInput")
    cst_d = nc.dram_tensor("consts", [128, CW], F32, kind="ExternalInput")

    if dbg_out:
        dbg_d = {
            "dbg_loss": nc.dram_tensor("dbg_loss", [NG, 1, 4], F32, kind="ExternalOutput"),
            "dbg_m": nc.dram_tensor("dbg_m", [NG, 1, 4], F32, kind="ExternalOutput"),
            "dbg_sg": nc.dram_tensor("dbg_sg", [NG, 1, 4], F32, kind="ExternalOutput"),
            "dbg_rp1": nc.dram_tensor("dbg_rp1", [NG, 1, 256], F32, kind="ExternalOutput"),
            "dbg_repl": nc.dram_tensor("dbg_repl", [NG, 1, 256], F32, kind="ExternalOutput"),
            "dbg_sidx": nc.dram_tensor("dbg_sidx", [NG, 1, 256], F32, kind="ExternalOutput"),
            "dbg_racs": nc.dram_tensor("dbg_racs", [NG, 64, 4], F32, kind="ExternalOutput"),
            "dbg_idxf": nc.dram_tensor("dbg_idxf", [NG, 128, 4], F32, kind="ExternalOutput"),
        }
    out_img = nc.dram_tensor("out_img", [SPB, 3, IMG, IMG], F32, kind="ExternalOutput")
    out_lab = nc.dram_tensor("out_lab", [SPB, IMG, IMG], I32, kind="ExternalOutput")
    out_cnf = nc.dram_tensor("out_cnf", [SPB, IMG, IMG], F32, kind="ExternalOutput")

    def patch_view(ap):
        # [ns, 256, 256] -> [ns, 32, 8, 256]  (r, bi, (bj w)) per sample
        return ap.rearrange("s (bi r) (bj w) -> s r bi (bj w)", bi=NH, bj=NH)

    with tile.TileContext(nc) as tc:
        with (
            tc.tile_pool(name="cstp", bufs=1) as cstp,
            tc.tile_pool(name="predp", bufs=1) as predp,
            tc.tile_pool(name="dice", bufs=2) as dice,
            tc.tile_pool(name="dice_e", bufs=3) as dice_e,
            tc.tile_pool(name="parts", bufs=3 * NG) as parts,
            tc.tile_pool(name="keys", bufs=2) as keys,
            tc.tile_pool(name="tiny", bufs=8) as tiny,
            tc.tile_pool(name="chin", bufs=4) as chin,
            tc.tile_pool(name="chout", bufs=2) as chout,
            tc.tile_pool(name="ps512", bufs=3, space="PSUM") as ps512,
            tc.tile_pool(name="ps256", bufs=2, space="PSUM") as ps256,
            tc.tile_pool(name="pstiny", bufs=2, space="PSUM") as pstiny,
        ):
            ct = cstp.tile([128, CW], F32, tag="ct")
            nc.sync.dma_start(ct, cst_d[:])
            cls_ones4 = ct[:, CO_BC32:CO_BC32 + 128]
            c_col = ct[:, CO_CCOL:CO_CCOL + 1]
            rg8 = ct[:, CO_RG8:CO_RG8 + 16]
            rg4 = ct[0:16, CO_RG4:CO_RG4 + 4]
            ones64 = ct[0:64, CO_ONES:CO_ONES + 64]
            qiota = ct[0:64, CO_QIOTA:CO_QIOTA + 1]
            piota = ct[0:1, CO_PIOTA:CO_PIOTA + 64]
            wsel = ct[0:64, CO_WSEL:CO_WSEL + 16]
            wmask4 = ct[0:64, CO_WM4:CO_WM4 + 16]
            smask = ct[:, CO_SMASK:CO_SMASK + 16]
            pisel = ct[0:16, CO_PISEL:CO_PISEL + 128]
            id4 = ct[0:4, CO_ID4:CO_ID4 + 4]
            id1 = ct[0:1, CO_ID1:CO_ID1 + 1]
            cls_grp = ct[:, CO_CGRP:CO_CGRP + 4]
            blkmask = ct[0:4, CO_BLK:CO_BLK + 256]
            bc32 = ct[0:64, CO_BC32:CO_BC32 + 128]
            # bf16 copy for the label-broadcast matmuls (labels exact 0..3)
            bc32h = cstp.tile([64, 128], BF16, tag="bc32h")
            nc.vector.tensor_copy(bc32h, bc32)

            idxts = []
            cfins = []
            for g in range(NG):
                s0 = g * GS

                # ---------------- dice loss ----------------
                scp = parts.tile([128, 16], F32, tag="part")
                intp = parts.tile([128, 16], F32, tag="part")
                cntp = parts.tile([128, 16], F32, tag="part")
                labv = olab_d[:].rearrange("s (a b) w -> s a (b w)", a=32)

                for hp in range(2):
                    e_ts = []
                    for sj in range(2):
                        si = 2 * hp + sj
                        pt = predp.tile([128, 2048], F32, tag="pred")
                        nc.sync.dma_start(
                            pt, pred_d[s0 + si].rearrange(
                                "c (a b) w -> (c a) (b w)", a=32))
                        et = dice_e.tile([128, 2048], F32, tag="e")
                        nc.scalar.activation(et, pt, AF.Exp)
                        e_ts.append(et)
                    # labels pair tile as bf16 [64=(s2,ph), 2048]
                    lf = dice.tile([64, 2048], BF16, tag="labf")
                    nc.gpsimd.dma_start(
                        out=lf, in_=labv[s0 + 2 * hp:s0 + 2 * hp + 2])

                    for sj in range(2):
                        si = 2 * hp + sj
                        psl = slice(32 * sj, 32 * sj + 32)
                        # r = exp(-ln(S)) replicated over class groups
                        rrep = dice.tile([128, 2048], F32, tag="rrep")
                        for ch in range(4):
                            cs = slice(ch * 512, ch * 512 + 512)
                            sps = ps512.tile([128, 512], F32, space="PSUM",
                                             tag="pb")
                            nc.tensor.matmul(sps, lhsT=cls_ones4,
                                             rhs=e_ts[sj][:, cs],
                                             start=True, stop=True)
                            lnt = dice.tile([128, 512], F32, tag="lnS")
                            nc.scalar.activation(lnt, sps, AF.Ln)
                            nc.scalar.activation(rrep[:, cs], lnt, AF.Exp,
                                                 scale=-1.0)
                        # s_c = e * r ; accum -> sum_s partials (per chunk)
                        scx = dice.tile([128, 2048], F32, tag="scx")
                        for ch in range(4):
                            cs = slice(ch * 512, ch * 512 + 512)
                            col = slice(4 * si + ch, 4 * si + ch + 1)
                            nc.vector.affine_mul_reduce(
                                scx[:, cs], scp[:, col],
                                e_ts[sj][:, cs], rrep[:, cs], 1.0, 0.0)
                        for ch in range(4):
                            cs = slice(ch * 512, ch * 512 + 512)
                            col = slice(4 * si + ch, 4 * si + ch + 1)
                            lbp = ps512.tile([128, 512], F32, space="PSUM",
                                             tag="pb")
                            nc.tensor.matmul(lbp, lhsT=bc32h[psl, :],
                                             rhs=lf[psl, cs],
                                             start=True, stop=True)
                            # count: mask with accum (junk out tile)
                            mjk = dice.tile([128, 512], F32, tag="mjk")
                            nc.any.tensor_scalar(mjk, lbp, c_col, None,
                                                 AL.is_equal, AL.add,
                                                 accum_out=cntp[:, col])
                            # inter = mask * s_c ; accum -> inter partial
                            nc.vector.scalar_tensor_tensor(
                                scx[:, cs], lbp, c_col, scx[:, cs],
                                AL.is_equal, AL.mult, accum_out=intp[:, col])

                # reduce partials -> per (class, sample) -> loss row
                q3 = []
                for ptile in (scp, intp, cntp):
                    pq = pstiny.tile([4, 16], F32, space="PSUM", tag="tp")
                    nc.tensor.matmul(pq, lhsT=cls_grp, rhs=ptile,
                                     start=True, stop=True)
                    q = tiny.tile([4, 4], F32, tag="q44")
                    nc.vector.tensor_reduce(q, pq.rearrange("c (s h) -> c s h", s=4),
                                            axis=mybir.AxisListType.X, op=AL.add)
                    q3.append(q)
                ss4, int4, cnt4 = q3
                u = tiny.tile([4, 4], F32, tag="q44")
                nc.vector.scalar_tensor_tensor(u, ss4, 1e-5, cnt4, AL.add, AL.add)
                iu = tiny.tile([4, 4], F32, tag="q44")
                nc.vector.reciprocal(iu, u)
                d4 = tiny.tile([4, 4], F32, tag="q44")
                nc.vector.scalar_tensor_tensor(d4, int4, 2.0, iu, AL.mult, AL.mult)
                lsump = pstiny.tile([1, 4], F32, space="PSUM", tag="tp")
                nc.tensor.matmul(lsump, lhsT=ones64[0:4, 0:1], rhs=d4,
                                 start=True, stop=True)
                loss = tiny.tile([1, 4], F32, tag="row4")
                nc.vector.tensor_scalar(loss, lsump, -1.0 / NCLS, 1.0,
                                        AL.mult, AL.add)
                trow = tiny.tile([1, 4], F32, tag="row4")
                nc.vector.tensor_scalar(trow, loss, -16.0 / AGE_DIV, 16.0,
                                        AL.mult, AL.add)
                ta = tiny.tile([1, 4], F32, tag="row4")
                nc.scalar.activation(ta, trow, AF.Abs)
                mrow = tiny.tile([1, 4], F32, tag="row4")
                nc.vector.tensor_scalar(mrow, ta, 16.0, None, AL.min)
                # sg01 = [loss < 1]  (1 when the self-paced branch flips sort)
                sgrow = tiny.tile([1, 4], F32, tag="row4")
                nc.vector.tensor_scalar(sgrow, loss, 1.0, None, AL.is_lt)

                # -------- keys: conf patch sums + sign-free ranks --------
                cfin = chin.tile([128, 128 * 32], F32, tag="cin")
                for si in range(GS):
                    row = slice(32 * si, 32 * si + 32)
                    nc.sync.dma_start(cfin[row, 0:2048],
                                      patch_view(ocnf_d[:])[s0 + si])
                    nc.sync.dma_start(cfin[row, 2048:4096],
                                      patch_view(acnf_d[:])[s0 + si])
                cfins.append(cfin)
                colred = keys.tile([128, 128], F32, tag="colred")
                nc.vector.tensor_reduce(colred,
                                        cfin[:].rearrange("p (n d) -> p n d", d=32),
                                        axis=mybir.AxisListType.X, op=AL.add)
                st1p = pstiny.tile([16, 128], F32, space="PSUM", tag="tp")
                nc.tensor.matmul(st1p, lhsT=rg8, rhs=colred, start=True, stop=True)
                st1 = keys.tile([16, 128], F32, tag="st1")
                nc.scalar.copy(st1, st1p)
                ksump = pstiny.tile([4, 128], F32, space="PSUM", tag="tp")
                nc.tensor.matmul(ksump, lhsT=rg4, rhs=st1, start=True, stop=True)
                kT = keys.tile([4, 128], F32, tag="kT")
                nc.scalar.copy(kT, ksump)
                kcp = pstiny.tile([128, 4], F32, space="PSUM", tag="tp")
                nc.tensor.transpose(kcp, kT, id4)
                kcols = keys.tile([128, 4], F32, tag="kcols")
                nc.scalar.copy(kcols, kcp)

                # block-diagonal expansion: rhs[k,(si,p)] = kT[k,p]*[si==k]
                blk3 = blkmask.rearrange("k (s p) -> k s p", s=4)
                kTeo = keys.tile([4, 256], F32, tag="kTe")
                nc.vector.tensor_tensor(
                    kTeo[:].rearrange("k (s p) -> k s p", s=4),
                    _bcast_free(kT[0:4, 0:64], [(1, 4)]), blk3, op=AL.mult)
                kTea = keys.tile([4, 256], F32, tag="kTe")
                nc.vector.tensor_tensor(
                    kTea[:].rearrange("k (s p) -> k s p", s=4),
                    _bcast_free(kT[0:4, 64:128], [(1, 4)]), blk3, op=AL.mult)
                obc = ps256.tile([64, 256], F32, space="PSUM", tag="k2")
                abc = ps256.tile([64, 256], F32, space="PSUM", tag="k2")
                nc.tensor.matmul(obc, lhsT=ones64[0:4, :], rhs=kTeo,
                                 start=True, stop=True)
                nc.tensor.matmul(abc, lhsT=ones64[0:4, :], rhs=kTea,
                                 start=True, stop=True)
                # A+[q,(s,p)] = [raw_key_s[p] > raw_key_s[q]]
                Ao = keys.tile([64, 256], F32, tag="Ao")
                nc.vector.tensor_tensor(
                    Ao, obc, _bcast_free(kcols[0:64, :], [(2, 64)]), op=AL.is_gt)
                Aa = keys.tile([64, 256], F32, tag="Aa")
                nc.vector.tensor_tensor(
                    Aa, abc, _bcast_free(kcols[64:128, :], [(2, 64)]), op=AL.is_gt)
                # ascending ranks of raw sums
                rorp = ps256.tile([64, 256], F32, space="PSUM", tag="k2")
                racp = pstiny.tile([64, 4], F32, space="PSUM", tag="tp")
                for si in range(GS):
                    osl = slice(64 * si, 64 * si + 64)
                    nc.tensor.matmul(rorp[:, osl], lhsT=ones64, rhs=Ao[:, osl],
                                     start=True, stop=True)
                    nc.tensor.matmul(racp[:, si:si + 1], lhsT=Aa[:, osl],
                                     rhs=ones64[:, 0:1], start=True, stop=True)
                rors = keys.tile([64, 256], F32, tag="rors")
                nc.scalar.copy(rors, rorp)
                racs = keys.tile([64, 4], F32, tag="racs")
                nc.scalar.copy(racs, racp)

                # -------- fixup with sign + build gather index --------
                # sg broadcast [64, (s)] and [64, (s, p)]
                sgp = pstiny.tile([64, 4], F32, space="PSUM", tag="tp")
                nc.tensor.matmul(sgp, lhsT=ones64[0:1, :], rhs=sgrow,
                                 start=True, stop=True)
                sgb = keys.tile([64, 4], F32, tag="sgb")
                nc.scalar.copy(sgb, sgp)
                # rank_o = r+ + sg*(63 - 2 r+)   (descending when sg=1)
                rob = keys.tile([64, 256], F32, tag="rob")
                nc.vector.tensor_scalar(rob, rors, -2.0, 63.0, AL.mult, AL.add)
                nc.vector.tensor_tensor(
                    rob, rob, _bcast_free(sgb[:, :], [(2, 64)]), op=AL.mult)
                nc.vector.tensor_tensor(rob, rob, rors, op=AL.add)
                # rank_a = r_a+ + (1-sg)*(63 - 2 r_a+)
                rac2 = keys.tile([64, 4], F32, tag="rac2")
                nc.vector.tensor_scalar(rac2, racs, -2.0, 63.0, AL.mult, AL.add)
                sga = keys.tile([64, 4], F32, tag="sga")
                nc.vector.tensor_scalar(sga, sgb, -1.0, 1.0, AL.mult, AL.add)
                nc.vector.tensor_tensor(rac2, rac2, sga, op=AL.mult)
                nc.vector.tensor_tensor(rac2, rac2, racs, op=AL.add)

                # M2[q,(s,p)] = [rank_a[q] == rank_o[p]] ; src = q @ M2
                M2 = keys.tile([64, 256], F32, tag="M2")
                nc.vector.tensor_tensor(
                    M2, rob, _bcast_free(rac2[:, :], [(2, 64)]), op=AL.is_equal)
                srcp = pstiny.tile([1, 256], F32, space="PSUM", tag="tp")
                nc.tensor.matmul(srcp, lhsT=qiota, rhs=M2, start=True, stop=True)

                rp1 = keys.tile([1, 256], F32, tag="rowt")
                nc.vector.tensor_scalar(rp1, rob[0:1, :], 1.0, None, AL.add)
                repl = keys.tile([1, 256], F32, tag="rowt")
                nc.vector.tensor_tensor(
                    repl, rp1, _bcast_free(mrow[0:1, :], [(2, 64)]), op=AL.is_le)
                x1 = keys.tile([1, 256], F32, tag="rowt")
                pio = _bcast_free(piota, [(1, 4)])
                nc.vector.tensor_tensor(x1, srcp, pio, op=AL.subtract)
                nc.vector.tensor_scalar(x1, x1, 64.0, None, AL.add)
                nc.vector.tensor_tensor(x1, x1, repl, op=AL.mult)
                nc.vector.tensor_tensor(x1, x1, pio, op=AL.add)

                sicp = pstiny.tile([64, 4], F32, space="PSUM", tag="tp")
                for si in range(GS):
                    nc.tensor.transpose(sicp[:, si:si + 1],
                                        x1[0:1, 64 * si:64 * si + 64], id1)
                sics = keys.tile([64, 4], F32, tag="sics")
                nc.scalar.copy(sics, sicp)
                # w1g[p, 4s+phi] = sidx_s[p] * [p//16 == phi]
                w1g = keys.tile([64, 16], F32, tag="w1g")
                nc.vector.tensor_tensor(
                    w1g[:].rearrange("p (s f) -> p s f", s=4),
                    _bcast_free(sics[:, :], [(2, 4)]),
                    wmask4.rearrange("p (s f) -> p s f", s=4), op=AL.mult)
                # wig[pi, 4s+phi] = sidx_s[16 phi + pi]
                wig = pstiny.tile([16, 16], F32, space="PSUM", tag="tp")
                nc.tensor.matmul(wig, lhsT=wsel, rhs=w1g, start=True, stop=True)
                wigs = keys.tile([16, 16], F32, tag="wigs")
                nc.scalar.copy(wigs, wig)
                # replicate rows to all 128 partitions, select own sample block
                wbig = ps256.tile([128, 16], F32, space="PSUM", tag="k2")
                nc.tensor.matmul(wbig, lhsT=pisel, rhs=wigs, start=True, stop=True)
                wsl = keys.tile([128, 16], F32, tag="wsl")
                nc.vector.tensor_tensor(wsl, wbig, smask, op=AL.mult)
                idxf = keys.tile([128, 4], F32, tag="idxf")
                ws_v = bass.AP(wsl[:].tensor, wsl[:].offset,
                               [list(wsl[:].ap[0]), [1, 4], [4, 4]])
                nc.vector.tensor_reduce(idxf, ws_v,
                                        axis=mybir.AxisListType.X, op=AL.add)
                idxt = keys.tile([128, 4], I16, tag="idxt")
                nc.vector.tensor_copy(idxt, idxf)
                idxts.append(idxt)

                if dbg_out:
                    for nm, t in [("dbg_loss", loss), ("dbg_m", mrow),
                                  ("dbg_sg", sgrow), ("dbg_rp1", rp1),
                                  ("dbg_repl", repl), ("dbg_sidx", x1),
                                  ("dbg_racs", rac2), ("dbg_idxf", idxf)]:
                        nc.sync.dma_start(dbg_d[nm][g], t)

            # ---------------- gather + write back, all chunks ----------------
            for g in range(NG):
                s0 = g * GS
                idxt = idxts[g]

                def store_chunk(cout, dstv):
                    for si in range(GS):
                        nc.sync.dma_start(
                            dstv[s0 + si],
                            cout[32 * si:32 * si + 32, :].rearrange(
                                "r (bi f) -> r bi f", bi=NH))

                def mix_chunk(osrcv, asrcv, dstv, dt):
                    cin = chin.tile([128, 128 * 32], dt, tag="cin")
                    for si in range(GS):
                        row = slice(32 * si, 32 * si + 32)
                        nc.sync.dma_start(cin[row, 0:2048], osrcv[s0 + si])
                        nc.sync.dma_start(cin[row, 2048:4096], asrcv[s0 + si])
                    cout = chout.tile([128, 64 * 32], dt, tag="cout")
                    nc.gpsimd.ap_gather(
                        cout[:].rearrange("p (n d) -> p n d", d=32),
                        cin[:].rearrange("p (n d) -> p n d", d=32),
                        idxt[:],
                        channels=128, num_elems=128, d=32, num_idxs=64)
                    store_chunk(cout, dstv)

                # conf chunk reuses the keys input tile (already loaded)
                ccout = chout.tile([128, 64 * 32], F32, tag="cout")
                nc.gpsimd.ap_gather(
                    ccout[:].rearrange("p (n d) -> p n d", d=32),
                    cfins[g][:].rearrange("p (n d) -> p n d", d=32),
                    idxt[:],
                    channels=128, num_elems=128, d=32, num_idxs=64)
                store_chunk(ccout, patch_view(out_cnf[:]))
                for c in range(3):
                    mix_chunk(patch_view(oimg_d[:, c]), patch_view(aimg_d[:, c]),
                              patch_view(out_img[:, c]), F32)
                mix_chunk(patch_view(olab_d[:]), patch_view(alab_d[:]),
                          patch_view(out_lab[:]), I32)

    nc.compile()
    return nc


_cached = {}


def kernel(oimage, aimage, olabel, alabel, oconf, aconf, prediction,
           cur_step=None):
    if "nc" not in _cached:
        _cached["nc"] = build_core_kernel(debug=False)
    nc = _cached["nc"]
    cst = _build_consts()
    in_maps = []
    for i in range(N_CORES):
        sl = slice(i * SPB, (i + 1) * SPB)
        in_maps.append({
            "pred": np.ascontiguousarray(prediction[sl], np.float32),
            "oimg": np.ascontiguousarray(oimage[sl], np.float32),
            "aimg": np.ascontiguousarray(aimage[sl], np.float32),
            "olab": np.ascontiguousarray(olabel[sl], np.int32),
            "alab": np.ascontiguousarray(alabel[sl], np.int32),
            "ocnf": np.ascontiguousarray(oconf[sl], np.float32),
            "acnf": np.ascontiguousarray(aconf[sl], np.float32),
            "consts": cst,
        })
    res = run_bass_kernel_spmd(nc, in_maps, core_ids=list(range(N_CORES)))
    _exec_info["exec_time_ns"] = res.exec_time_ns
    img = np.concatenate([res.results[i]["out_img"] for i in range(N_CORES)])
    lab = np.concatenate([res.results[i]["out_lab"] for i in range(N_CORES)])
    cnf = np.concatenate([res.results[i]["out_cnf"] for i in range(N_CORES)])
    return img, lab.astype(np.int32), cnf


# revision 45
# speedup vs baseline: 1.7472x; 1.1275x over previous
"""AdaMix forward on 8 Trainium2 NeuronCores (Bass/Tile), pure data parallel.

Per core: 8 samples, processed as 2 groups of 4. Phases (heavily pipelined by
Tile): per group -> dice loss (classes-on-partitions, replicated class-sum
matmul, r = exp(-ln(S)) on ScalarE, fused AMR/STT accumulation), conf patch
sums + sign-independent ranks (all-pairs compares + one-hot matmuls), tiny
sign/k fixup -> per-sample gather index; then all channel chunks stream
through gpsimd ap_gather (o patches = slots 0..63, a = 64..127).
"""

import os
import numpy as np

import concourse.bass as bass
import concourse.mybir as mybir
import concourse.tile as tile
from concourse import bacc
from concourse.bass_utils import run_bass_kernel_spmd

F32 = mybir.dt.float32
BF16 = mybir.dt.bfloat16
I32 = mybir.dt.int32
I16 = mybir.dt.int16
AL = mybir.AluOpType
AF = mybir.ActivationFunctionType

N_CORES = 8
B = 64
SPB = B // N_CORES      # samples per core
GS = 4                  # samples per group
NG = SPB // GS          # groups per core
NCLS = 4
IMG = 256
PS = 32                 # patch side
NH = 8                  # patches per image side
NP = 64                 # patches per image
AGE_DIV = 1.0 + 1e-5

# ---------------- constant pack (one [128, CW] f32 tensor) ----------------
CO_CLS = 0      # [128,32]  cls_ones:  k%32==m   (class sum over c)
CO_BC32 = 32    # [128,128] bc32:      m%32==k%32 (replicated broadcast)
CO_CCOL = 160   # [128,1]   c_col:     k//32
CO_RG8 = 161    # [128,16]  rgrp8:     k//8==m
CO_RG4 = 177    # [16,4]    rgrp4:     k//4==m
CO_ONES = 181   # [64,64]   ones
CO_QIOTA = 245  # [64,1]    qiota:     k
CO_PIOTA = 246  # [1,64]    piota:     m
CO_WMASK = 310  # [64,4]    (unused)
CO_WSEL = 314   # [64,16]   wrapsel:   k%16==m
CO_ID4 = 330    # [4,4]     identity
CO_ID1 = 334    # [1,1]     identity
CO_CGRP = 335   # [128,4]   cls_grp:   k//32==m
CO_BLK = 339    # [4,256]   blkmask:   k==m//64
CO_WM4 = 595    # [64,16]   wmask4:    k//16==m%4
CO_SMASK = 611  # [128,16]  smask:     k//32==m//4
CO_PISEL = 627  # [16,128]  pisel:     k==m%16
CW = 756

_exec_info = {}


def _install_ntff_hook():
    """The agent image's antenv lacks axon_hooks; rebuild it from the boot
    helpers so trace=True (BASS_TRACE=1) works for profiling."""
    import sys
    import types
    try:
        import antenv.axon_hooks  # noqa: F401
        return
    except ImportError:
        pass
    try:
        import antenv
        from trn_agent_boot.trn_boot import _ntff_profile_via_ctypes
        hook = _ntff_profile_via_ctypes("/opt/axon/libaxon_pjrt.so")
        mod = types.ModuleType("antenv.axon_hooks")
        state = {"hook": hook}
        mod.set_axon_ntff_profile_hook = lambda h: state.update(hook=h)
        mod.get_axon_ntff_profile_hook = lambda: state["hook"]
        sys.modules["antenv.axon_hooks"] = mod
        antenv.axon_hooks = mod
    except Exception:
        pass


_install_ntff_hook()


def _build_consts() -> np.ndarray:
    ct = np.zeros((128, CW), np.float32)
    k = np.arange(128)
    ct[:, CO_CLS:CO_CLS + 32] = (k[:, None] % 32 == np.arange(32)[None, :])
    ct[:, CO_BC32:CO_BC32 + 128] = (np.arange(128)[None, :] % 32 == k[:, None] % 32)
    ct[:, CO_CCOL] = k // 32
    ct[:, CO_RG8:CO_RG8 + 16] = (k[:, None] // 8 == np.arange(16)[None, :])
    ct[:16, CO_RG4:CO_RG4 + 4] = (k[:16, None] // 4 == np.arange(4)[None, :])
    ct[:64, CO_ONES:CO_ONES + 64] = 1.0
    ct[:64, CO_QIOTA] = k[:64]
    ct[0, CO_PIOTA:CO_PIOTA + 64] = np.arange(64)
    ct[:64, CO_WSEL:CO_WSEL + 16] = (k[:64, None] % 16 == np.arange(16)[None, :])
    ct[:4, CO_ID4:CO_ID4 + 4] = np.eye(4)
    ct[0, CO_ID1] = 1.0
    ct[:, CO_CGRP:CO_CGRP + 4] = (k[:, None] // 32 == np.arange(4)[None, :])
    ct[:4, CO_BLK:CO_BLK + 256] = (k[:4, None] == np.arange(256)[None, :] // 64)
    ct[:64, CO_WM4:CO_WM4 + 16] = (k[:64, None] // 16 == np.arange(16)[None, :] % 4)
    ct[:, CO_SMASK:CO_SMASK + 16] = (k[:, None] // 32 == np.arange(16)[None, :] // 4)
    ct[:16, CO_PISEL:CO_PISEL + 128] = (k[:16, None] == np.arange(128)[None, :] % 16)
    return ct


def _bcast_free(ap: bass.AP, dims) -> bass.AP:
    """Insert stride-0 free dims at the given (pos, count) positions."""
    new = [list(p) for p in ap.ap]
    for pos, count in dims:
        new.insert(pos, [0, count])
    return bass.AP(ap.tensor, ap.offset, new)


def build_core_kernel(debug: bool = False, dbg_out: bool = False) -> bacc.Bacc:
    nc = bacc.Bacc("TRN2", target_bir_lowering=False, debug=debug,
                   num_devices=N_CORES)

    pred_d = nc.dram_tensor("pred", [SPB, NCLS, IMG, IMG], F32, kind="ExternalInput")
    oimg_d = nc.dram_tensor("oimg", [SPB, 3, IMG, IMG], F32, kind="ExternalInput")
    aimg_d = nc.dram_tensor("aimg", [SPB, 3, IMG, IMG], F32, kind="ExternalInput")
    olab_d = nc.dram_tensor("olab", [SPB, IMG, IMG], I32, kind="ExternalInput")
    alab_d = nc.dram_tensor("alab", [SPB, IMG, IMG], I32, kind="ExternalInput")
    ocnf_d = nc.dram_tensor("ocnf", [SPB, IMG, IMG], F32, kind="ExternalInput")
    acnf_d = nc.dram_tensor("acnf", [SPB, IMG, IMG], F32, kind="ExternalInput")
    cst_d = nc.dram_tensor("consts", [128, CW], F32, kind="ExternalInput")

    if dbg_out:
        dbg_d = {
            "dbg_loss": nc.dram_tensor("dbg_loss", [NG, 1, 4], F32, kind="ExternalOutput"),
            "dbg_m": nc.dram_tensor("dbg_m", [NG, 1, 4], F32, kind="ExternalOutput"),
            "dbg_sg": nc.dram_tensor("dbg_sg", [NG, 1, 4], F32, kind="ExternalOutput"),
            "dbg_rp1": nc.dram_tensor("dbg_rp1", [NG, 1, 256], F32, kind="ExternalOutput"),
            "dbg_repl": nc.dram_tensor("dbg_repl", [NG, 1, 256], F32, kind="ExternalOutput"),
            "dbg_sidx": nc.dram_tensor("dbg_sidx", [NG, 1, 256], F32, kind="ExternalOutput"),
            "dbg_racs": nc.dram_tensor("dbg_racs", [NG, 64, 4], F32, kind="ExternalOutput"),
            "dbg_idxf": nc.dram_tensor("dbg_idxf", [NG, 128, 4], F32, kind="ExternalOutput"),
        }
    out_img = nc.dram_tensor("out_img", [SPB, 3, IMG, IMG], F32, kind="ExternalOutput")
    out_lab = nc.dram_tensor("out_lab", [SPB, IMG, IMG], I32, kind="ExternalOutput")
    out_cnf = nc.dram_tensor("out_cnf", [SPB, IMG, IMG], F32, kind="ExternalOutput")

    def patch_view(ap):
        # [ns, 256, 256] -> [ns, 32, 8, 256]  (r, bi, (bj w)) per sample
        return ap.rearrange("s (bi r) (bj w) -> s r bi (bj w)", bi=NH, bj=NH)

    with tile.TileContext(nc) as tc:
        with (
            tc.tile_pool(name="cstp", bufs=1) as cstp,
            tc.tile_pool(name="predp", bufs=1) as predp,
            tc.tile_pool(name="dice", bufs=2) as dice,
            tc.tile_pool(name="dice_e", bufs=2) as dice_e,
            tc.tile_pool(name="parts", bufs=3 * NG) as parts,
            tc.tile_pool(name="keys", bufs=2) as keys,
            tc.tile_pool(name="tiny", bufs=8) as tiny,
            tc.tile_pool(name="chin", bufs=4) as chin,
            tc.tile_pool(name="chout", bufs=2) as chout,
            tc.tile_pool(name="ps512", bufs=4, space="PSUM") as ps512,
            tc.tile_pool(name="ps256", bufs=2, space="PSUM") as ps256,
            tc.tile_pool(name="pstiny", bufs=2, space="PSUM") as pstiny,
        ):
            ct = cstp.tile([128, CW], F32, tag="ct")
            nc.sync.dma_start(ct, cst_d[:])
            cls_ones4 = ct[:, CO_BC32:CO_BC32 + 128]
            c_col = ct[:, CO_CCOL:CO_CCOL + 1]
            rg8 = ct[:, CO_RG8:CO_RG8 + 16]
            rg4 = ct[0:16, CO_RG4:CO_RG4 + 4]
            ones64 = ct[0:64, CO_ONES:CO_ONES + 64]
            qiota = ct[0:64, CO_QIOTA:CO_QIOTA + 1]
            piota = ct[0:1, CO_PIOTA:CO_PIOTA + 64]
            wsel = ct[0:64, CO_WSEL:CO_WSEL + 16]
            wmask4 = ct[0:64, CO_WM4:CO_WM4 + 16]
            smask = ct[:, CO_SMASK:CO_SMASK + 16]
            pisel = ct[0:16, CO_PISEL:CO_PISEL + 128]
            id4 = ct[0:4, CO_ID4:CO_ID4 + 4]
            id1 = ct[0:1, CO_ID1:CO_ID1 + 1]
            cls_grp = ct[:, CO_CGRP:CO_CGRP + 4]
            blkmask = ct[0:4, CO_BLK:CO_BLK + 256]
            bc32 = ct[0:64, CO_BC32:CO_BC32 + 128]
            # bf16 copy for the label-broadcast matmuls (labels exact 0..3)
            bc32h = cstp.tile([64, 128], BF16, tag="bc32h")
            nc.vector.tensor_copy(bc32h, bc32)

            idxts = []
            cfins = []
            for g in range(NG):
                s0 = g * GS

                # ---------------- dice loss ----------------
                scp = parts.tile([128, 16], F32, tag="part")
                intp = parts.tile([128, 16], F32, tag="part")
                cntp = parts.tile([128, 16], F32, tag="part")
                labv = olab_d[:].rearrange("s (a b) w -> s a (b w)", a=32)

                for hp in range(2):
                    e_ts = []
                    for sj in range(2):
                        si = 2 * hp + sj
                        pt = predp.tile([128, 2048], F32, tag="pred")
                        nc.sync.dma_start(
                            pt, pred_d[s0 + si].rearrange(
                                "c (a b) w -> (c a) (b w)", a=32))
                        et = dice_e.tile([128, 2048], F32, tag="e")
                        nc.scalar.activation(et, pt, AF.Exp)
                        e_ts.append(et)
                    # labels pair tile as bf16 [64=(s2,ph), 2048]
                    lf = dice.tile([64, 2048], BF16, tag="labf")
                    nc.gpsimd.dma_start(
                        out=lf, in_=labv[s0 + 2 * hp:s0 + 2 * hp + 2])

                    for sj in range(2):
                        si = 2 * hp + sj
                        psl = slice(32 * sj, 32 * sj + 32)
                        # r = exp(-ln(S)) replicated over class groups;
                        # batch Ln then Exp to avoid ACT table reloads
                        rrep = dice.tile([128, 2048], F32, tag="rrep")
                        spss = []
                        for ch in range(4):
                            cs = slice(ch * 512, ch * 512 + 512)
                            sps = ps512.tile([128, 512], F32, space="PSUM",
                                             tag="pb")
                            nc.tensor.matmul(sps, lhsT=cls_ones4,
                                             rhs=e_ts[sj][:, cs],
                                             start=True, stop=True)
                            spss.append(sps)
                        lnfull = dice.tile([128, 2048], F32, tag="lnfull")
                        for ch in range(4):
                            cs = slice(ch * 512, ch * 512 + 512)
                            nc.scalar.activation(lnfull[:, cs], spss[ch], AF.Ln)
                        nc.scalar.activation(rrep, lnfull, AF.Exp, scale=-1.0)
                        # s_c = e * r ; accum -> sum_s partials (per chunk)
                        scx = dice.tile([128, 2048], F32, tag="scx")
                        for ch in range(4):
                            cs = slice(ch * 512, ch * 512 + 512)
                            col = slice(4 * si + ch, 4 * si + ch + 1)
                            nc.vector.affine_mul_reduce(
                                scx[:, cs], scp[:, col],
                                e_ts[sj][:, cs], rrep[:, cs], 1.0, 0.0)
                        for ch in range(4):
                            cs = slice(ch * 512, ch * 512 + 512)
                            col = slice(4 * si + ch, 4 * si + ch + 1)
                            lbp = ps512.tile([128, 512], F32, space="PSUM",
                                             tag="pb")
                            nc.tensor.matmul(lbp, lhsT=bc32h[psl, :],
                                             rhs=lf[psl, cs],
                                             start=True, stop=True)
                            # count: mask with accum (junk out tile)
                            mjk = dice.tile([128, 512], F32, tag="mjk")
                            nc.any.tensor_scalar(mjk, lbp, c_col, None,
                                                 AL.is_equal, AL.add,
                                                 accum_out=cntp[:, col])
                            # inter = mask * s_c ; accum -> inter partial
                            nc.vector.scalar_tensor_tensor(
                                scx[:, cs], lbp, c_col, scx[:, cs],
                                AL.is_equal, AL.mult, accum_out=intp[:, col])

                # reduce partials -> per (class, sample) -> loss row
                q3 = []
                for ptile in (scp, intp, cntp):
                    pq = pstiny.tile([4, 16], F32, space="PSUM", tag="tp")
                    nc.tensor.matmul(pq, lhsT=cls_grp, rhs=ptile,
                                     start=True, stop=True)
                    q = tiny.tile([4, 4], F32, tag="q44")
                    nc.vector.tensor_reduce(q, pq.rearrange("c (s h) -> c s h", s=4),
                                            axis=mybir.AxisListType.X, op=AL.add)
                    q3.append(q)
                ss4, int4, cnt4 = q3
                u = tiny.tile([4, 4], F32, tag="q44")
                nc.vector.scalar_tensor_tensor(u, ss4, 1e-5, cnt4, AL.add, AL.add)
                iu = tiny.tile([4, 4], F32, tag="q44")
                nc.vector.reciprocal(iu, u)
                d4 = tiny.tile([4, 4], F32, tag="q44")
                nc.vector.scalar_tensor_tensor(d4, int4, 2.0, iu, AL.mult, AL.mult)
                lsump = pstiny.tile([1, 4], F32, space="PSUM", tag="tp")
                nc.tensor.matmul(lsump, lhsT=ones64[0:4, 0:1], rhs=d4,
                                 start=True, stop=True)
                loss = tiny.tile([1, 4], F32, tag="row4")
                nc.vector.tensor_scalar(loss, lsump, -1.0 / NCLS, 1.0,
                                        AL.mult, AL.add)
                trow = tiny.tile([1, 4], F32, tag="row4")
                nc.vector.tensor_scalar(trow, loss, -16.0 / AGE_DIV, 16.0,
                                        AL.mult, AL.add)
                tneg = tiny.tile([1, 4], F32, tag="row4")
                nc.vector.tensor_scalar(tneg, trow, -1.0, None, AL.mult)
                ta = tiny.tile([1, 4], F32, tag="row4")
                nc.vector.tensor_tensor(ta, trow, tneg, op=AL.max)
                mrow = tiny.tile([1, 4], F32, tag="row4")
                nc.vector.tensor_scalar(mrow, ta, 16.0, None, AL.min)
                # sg01 = [loss < 1]  (1 when the self-paced branch flips sort)
                sgrow = tiny.tile([1, 4], F32, tag="row4")
                nc.vector.tensor_scalar(sgrow, loss, 1.0, None, AL.is_lt)

                # -------- keys: conf patch sums + sign-free ranks --------
                cfin = chin.tile([128, 128 * 32], F32, tag="cin")
                for si in range(GS):
                    row = slice(32 * si, 32 * si + 32)
                    nc.sync.dma_start(cfin[row, 0:2048],
                                      patch_view(ocnf_d[:])[s0 + si])
                    nc.sync.dma_start(cfin[row, 2048:4096],
                                      patch_view(acnf_d[:])[s0 + si])
                cfins.append(cfin)
                colred = keys.tile([128, 128], F32, tag="colred")
                nc.vector.tensor_reduce(colred,
                                        cfin[:].rearrange("p (n d) -> p n d", d=32),
                                        axis=mybir.AxisListType.X, op=AL.add)
                st1p = pstiny.tile([16, 128], F32, space="PSUM", tag="tp")
                nc.tensor.matmul(st1p, lhsT=rg8, rhs=colred, start=True, stop=True)
                st1 = keys.tile([16, 128], F32, tag="st1")
                nc.vector.tensor_copy(st1, st1p)
                ksump = pstiny.tile([4, 128], F32, space="PSUM", tag="tp")
                nc.tensor.matmul(ksump, lhsT=rg4, rhs=st1, start=True, stop=True)
                kT = keys.tile([4, 128], F32, tag="kT")
                nc.vector.tensor_copy(kT, ksump)
                kcp = pstiny.tile([128, 4], F32, space="PSUM", tag="tp")
                nc.tensor.transpose(kcp, kT, id4)
                kcols = keys.tile([128, 4], F32, tag="kcols")
                nc.vector.tensor_copy(kcols, kcp)

                # block-diagonal expansion: rhs[k,(si,p)] = kT[k,p]*[si==k]
                blk3 = blkmask.rearrange("k (s p) -> k s p", s=4)
                kTeo = keys.tile([4, 256], F32, tag="kTe")
                nc.vector.tensor_tensor(
                    kTeo[:].rearrange("k (s p) -> k s p", s=4),
                    _bcast_free(kT[0:4, 0:64], [(1, 4)]), blk3, op=AL.mult)
                kTea = keys.tile([4, 256], F32, tag="kTe")
                nc.vector.tensor_tensor(
                    kTea[:].rearrange("k (s p) -> k s p", s=4),
                    _bcast_free(kT[0:4, 64:128], [(1, 4)]), blk3, op=AL.mult)
                obc = ps256.tile([64, 256], F32, space="PSUM", tag="k2")
                abc = ps256.tile([64, 256], F32, space="PSUM", tag="k2")
                nc.tensor.matmul(obc, lhsT=ones64[0:4, :], rhs=kTeo,
                                 start=True, stop=True)
                nc.tensor.matmul(abc, lhsT=ones64[0:4, :], rhs=kTea,
                                 start=True, stop=True)
                # A+[q,(s,p)] = [raw_key_s[p] > raw_key_s[q]]
                Ao = keys.tile([64, 256], F32, tag="Ao")
                nc.vector.tensor_tensor(
                    Ao, obc, _bcast_free(kcols[0:64, :], [(2, 64)]), op=AL.is_gt)
                Aa = keys.tile([64, 256], F32, tag="Aa")
                nc.vector.tensor_tensor(
                    Aa, abc, _bcast_free(kcols[64:128, :], [(2, 64)]), op=AL.is_gt)
                # ascending ranks of raw sums
                rorp = ps256.tile([64, 256], F32, space="PSUM", tag="k2")
                racp = pstiny.tile([64, 4], F32, space="PSUM", tag="tp")
                for si in range(GS):
                    osl = slice(64 * si, 64 * si + 64)
                    nc.tensor.matmul(rorp[:, osl], lhsT=ones64, rhs=Ao[:, osl],
                                     start=True, stop=True)
                    nc.tensor.matmul(racp[:, si:si + 1], lhsT=Aa[:, osl],
                                     rhs=ones64[:, 0:1], start=True, stop=True)
                rors = keys.tile([64, 256], F32, tag="rors")
                nc.vector.tensor_copy(rors, rorp)
                racs = keys.tile([64, 4], F32, tag="racs")
                nc.vector.tensor_copy(racs, racp)

                # -------- fixup with sign + build gather index --------
                # sg broadcast [64, (s)] and [64, (s, p)]
                sgp = pstiny.tile([64, 4], F32, space="PSUM", tag="tp")
                nc.tensor.matmul(sgp, lhsT=ones64[0:1, :], rhs=sgrow,
                                 start=True, stop=True)
                sgb = keys.tile([64, 4], F32, tag="sgb")
                nc.vector.tensor_copy(sgb, sgp)
                # rank_o = r+ + sg*(63 - 2 r+)   (descending when sg=1)
                rob = keys.tile([64, 256], F32, tag="rob")
                nc.vector.tensor_scalar(rob, rors, -2.0, 63.0, AL.mult, AL.add)
                nc.vector.tensor_tensor(
                    rob, rob, _bcast_free(sgb[:, :], [(2, 64)]), op=AL.mult)
                nc.vector.tensor_tensor(rob, rob, rors, op=AL.add)
                # rank_a = r_a+ + (1-sg)*(63 - 2 r_a+)
                rac2 = keys.tile([64, 4], F32, tag="rac2")
                nc.vector.tensor_scalar(rac2, racs, -2.0, 63.0, AL.mult, AL.add)
                sga = keys.tile([64, 4], F32, tag="sga")
                nc.vector.tensor_scalar(sga, sgb, -1.0, 1.0, AL.mult, AL.add)
                nc.vector.tensor_tensor(rac2, rac2, sga, op=AL.mult)
                nc.vector.tensor_tensor(rac2, rac2, racs, op=AL.add)

                # M2[q,(s,p)] = [rank_a[q] == rank_o[p]] ; src = q @ M2
                M2 = keys.tile([64, 256], F32, tag="M2")
                nc.vector.tensor_tensor(
                    M2, rob, _bcast_free(rac2[:, :], [(2, 64)]), op=AL.is_equal)
                srcp = pstiny.tile([1, 256], F32, space="PSUM", tag="tp")
                nc.tensor.matmul(srcp, lhsT=qiota, rhs=M2, start=True, stop=True)

                rp1 = keys.tile([1, 256], F32, tag="rowt")
                nc.vector.tensor_scalar(rp1, rob[0:1, :], 1.0, None, AL.add)
                repl = keys.tile([1, 256], F32, tag="rowt")
                nc.vector.tensor_tensor(
                    repl, rp1, _bcast_free(mrow[0:1, :], [(2, 64)]), op=AL.is_le)
                x1 = keys.tile([1, 256], F32, tag="rowt")
                pio = _bcast_free(piota, [(1, 4)])
                nc.vector.tensor_tensor(x1, srcp, pio, op=AL.subtract)
                nc.vector.tensor_scalar(x1, x1, 64.0, None, AL.add)
                nc.vector.tensor_tensor(x1, x1, repl, op=AL.mult)
                nc.vector.tensor_tensor(x1, x1, pio, op=AL.add)

                sicp = pstiny.tile([64, 4], F32, space="PSUM", tag="tp")
                for si in range(GS):
                    nc.tensor.transpose(sicp[:, si:si + 1],
                                        x1[0:1, 64 * si:64 * si + 64], id1)
                sics = keys.tile([64, 4], F32, tag="sics")
                nc.vector.tensor_copy(sics, sicp)
                # w1g[p, 4s+phi] = sidx_s[p] * [p//16 == phi]
                w1g = keys.tile([64, 16], F32, tag="w1g")
                nc.vector.tensor_tensor(
                    w1g[:].rearrange("p (s f) -> p s f", s=4),
                    _bcast_free(sics[:, :], [(2, 4)]),
                    wmask4.rearrange("p (s f) -> p s f", s=4), op=AL.mult)
                # wig[pi, 4s+phi] = sidx_s[16 phi + pi]
                wig = pstiny.tile([16, 16], F32, space="PSUM", tag="tp")
                nc.tensor.matmul(wig, lhsT=wsel, rhs=w1g, start=True, stop=True)
                wigs = keys.tile([16, 16], F32, tag="wigs")
                nc.vector.tensor_copy(wigs, wig)
                # replicate rows to all 128 partitions, select own sample block
                wbig = ps256.tile([128, 16], F32, space="PSUM", tag="k2")
                nc.tensor.matmul(wbig, lhsT=pisel, rhs=wigs, start=True, stop=True)
                wsl = keys.tile([128, 16], F32, tag="wsl")
                nc.vector.tensor_tensor(wsl, wbig, smask, op=AL.mult)
                idxf = keys.tile([128, 4], F32, tag="idxf")
                ws_v = bass.AP(wsl[:].tensor, wsl[:].offset,
                               [list(wsl[:].ap[0]), [1, 4], [4, 4]])
                nc.vector.tensor_reduce(idxf, ws_v,
                                        axis=mybir.AxisListType.X, op=AL.add)
                idxt = keys.tile([128, 4], I16, tag="idxt")
                nc.vector.tensor_copy(idxt, idxf)
                idxts.append(idxt)

                if dbg_out:
                    for nm, t in [("dbg_loss", loss), ("dbg_m", mrow),
                                  ("dbg_sg", sgrow), ("dbg_rp1", rp1),
                                  ("dbg_repl", repl), ("dbg_sidx", x1),
                                  ("dbg_racs", rac2), ("dbg_idxf", idxf)]:
                        nc.sync.dma_start(dbg_d[nm][g], t)

            # ---------------- gather + write back, all chunks ----------------
            def load_chunk(g, osrcv, asrcv, dt):
                s0 = g * GS
                cin = chin.tile([128, 128 * 32], dt, tag="cin")
                for si in range(GS):
                    row = slice(32 * si, 32 * si + 32)
                    nc.sync.dma_start(cin[row, 0:2048], osrcv[s0 + si])
                    nc.sync.dma_start(cin[row, 2048:4096], asrcv[s0 + si])
                return cin

            def gather_store(g, cin, dstv, dt):
                s0 = g * GS
                cout = chout.tile([128, 64 * 32], dt, tag="cout")
                nc.gpsimd.ap_gather(
                    cout[:].rearrange("p (n d) -> p n d", d=32),
                    cin[:].rearrange("p (n d) -> p n d", d=32),
                    idxts[g][:],
                    channels=128, num_elems=128, d=32, num_idxs=64)
                for si in range(GS):
                    nc.sync.dma_start(
                        dstv[s0 + si],
                        cout[32 * si:32 * si + 32, :].rearrange(
                            "r (bi f) -> r bi f", bi=NH))

            chunks = [
                (patch_view(oimg_d[:, 0]), patch_view(aimg_d[:, 0]),
                 patch_view(out_img[:, 0]), F32),
                (patch_view(oimg_d[:, 1]), patch_view(aimg_d[:, 1]),
                 patch_view(out_img[:, 1]), F32),
                (patch_view(oimg_d[:, 2]), patch_view(aimg_d[:, 2]),
                 patch_view(out_img[:, 2]), F32),
                (patch_view(olab_d[:]), patch_view(alab_d[:]),
                 patch_view(out_lab[:]), I32),
            ]
            # stream: conf gathers first (tiles already resident), then the
            # 4 remaining chunks per group, loads running ahead of gathers
            pend = []
            for g in range(NG):
                for (ov, av, dv, dt) in chunks:
                    pend.append((g, load_chunk(g, ov, av, dt), dv, dt))
            for g in range(NG):
                gather_store(g, cfins[g], patch_view(out_cnf[:]), F32)
            for (g, cin, dv, dt) in pend:
                gather_store(g, cin, dv, dt)

    nc.compile()
    return nc


_cached = {}


def kernel(oimage, aimage, olabel, alabel, oconf, aconf, prediction,
           cur_step=None):
    if "nc" not in _cached:
        _cached["nc"] = build_core_kernel(debug=False)
    nc = _cached["nc"]
    cst = _build_consts()
    in_maps = []
    for i in range(N_CORES):
        sl = slice(i * SPB, (i + 1) * SPB)
        in_maps.append({
            "pred": np.ascontiguousarray(prediction[sl], np.float32),
            "oimg": np.ascontiguousarray(oimage[sl], np.float32),
            "aimg": np.ascontiguousarray(aimage[sl], np.float32),
            "olab": np.ascontiguousarray(olabel[sl], np.int32),
            "alab": np.ascontiguousarray(alabel[sl], np.int32),
            "ocnf": np.ascontiguousarray(oconf[sl], np.float32),
            "acnf": np.ascontiguousarray(aconf[sl], np.float32),
            "consts": cst,
        })
    res = run_bass_kernel_spmd(nc, in_maps, core_ids=list(range(N_CORES)))
    _exec_info["exec_time_ns"] = res.exec_time_ns
    img = np.concatenate([res.results[i]["out_img"] for i in range(N_CORES)])
    lab = np.concatenate([res.results[i]["out_lab"] for i in range(N_CORES)])
    cnf = np.concatenate([res.results[i]["out_cnf"] for i in range(N_CORES)])
    return img, lab.astype(np.int32), cnf


# revision 49
# speedup vs baseline: 1.7586x; 1.0065x over previous
"""AdaMix forward on 8 Trainium2 NeuronCores (Bass/Tile), pure data parallel.

Per core: 8 samples, processed as 2 groups of 4. Phases (heavily pipelined by
Tile): per group -> dice loss (classes-on-partitions, replicated class-sum
matmul, r = exp(-ln(S)) on ScalarE, fused AMR/STT accumulation), conf patch
sums + sign-independent ranks (all-pairs compares + one-hot matmuls), tiny
sign/k fixup -> per-sample gather index; then all channel chunks stream
through gpsimd ap_gather (o patches = slots 0..63, a = 64..127).
"""

import os
import numpy as np

import concourse.bass as bass
import concourse.mybir as mybir
import concourse.tile as tile
from concourse import bacc
from concourse.bass_utils import run_bass_kernel_spmd

F32 = mybir.dt.float32
BF16 = mybir.dt.bfloat16
I32 = mybir.dt.int32
I16 = mybir.dt.int16
AL = mybir.AluOpType
AF = mybir.ActivationFunctionType

N_CORES = 8
B = 64
SPB = B // N_CORES      # samples per core
GS = 4                  # samples per group
NG = SPB // GS          # groups per core
NCLS = 4
IMG = 256
PS = 32                 # patch side
NH = 8                  # patches per image side
NP = 64                 # patches per image
AGE_DIV = 1.0 + 1e-5

# ---------------- constant pack (one [128, CW] f32 tensor) ----------------
CO_CLS = 0      # [128,32]  cls_ones:  k%32==m   (class sum over c)
CO_BC32 = 32    # [128,128] bc32:      m%32==k%32 (replicated broadcast)
CO_CCOL = 160   # [128,1]   c_col:     k//32
CO_RG8 = 161    # [128,16]  rgrp8:     k//8==m
CO_RG4 = 177    # [16,4]    rgrp4:     k//4==m
CO_ONES = 181   # [64,64]   ones
CO_QIOTA = 245  # [64,1]    qiota:     k
CO_PIOTA = 246  # [1,64]    piota:     m
CO_WMASK = 310  # [64,4]    (unused)
CO_WSEL = 314   # [64,16]   wrapsel:   k%16==m
CO_ID4 = 330    # [4,4]     identity
CO_ID1 = 334    # [1,1]     identity
CO_CGRP = 335   # [128,4]   cls_grp:   k//32==m
CO_BLK = 339    # [4,256]   blkmask:   k==m//64
CO_WM4 = 595    # [64,16]   wmask4:    k//16==m%4
CO_SMASK = 611  # [128,16]  smask:     k//32==m//4
CO_PISEL = 627  # [16,128]  pisel:     k==m%16
CW = 756

_exec_info = {}


def _install_ntff_hook():
    """The agent image's antenv lacks axon_hooks; rebuild it from the boot
    helpers so trace=True (BASS_TRACE=1) works for profiling."""
    import sys
    import types
    try:
        import antenv.axon_hooks  # noqa: F401
        return
    except ImportError:
        pass
    try:
        import antenv
        from trn_agent_boot.trn_boot import _ntff_profile_via_ctypes
        hook = _ntff_profile_via_ctypes("/opt/axon/libaxon_pjrt.so")
        mod = types.ModuleType("antenv.axon_hooks")
        state = {"hook": hook}
        mod.set_axon_ntff_profile_hook = lambda h: state.update(hook=h)
        mod.get_axon_ntff_profile_hook = lambda: state["hook"]
        sys.modules["antenv.axon_hooks"] = mod
        antenv.axon_hooks = mod
    except Exception:
        pass


_install_ntff_hook()


def _build_consts() -> np.ndarray:
    ct = np.zeros((128, CW), np.float32)
    k = np.arange(128)
    ct[:, CO_CLS:CO_CLS + 32] = (k[:, None] % 32 == np.arange(32)[None, :])
    ct[:, CO_BC32:CO_BC32 + 128] = (np.arange(128)[None, :] % 32 == k[:, None] % 32)
    ct[:, CO_CCOL] = k // 32
    ct[:, CO_RG8:CO_RG8 + 16] = (k[:, None] // 8 == np.arange(16)[None, :])
    ct[:16, CO_RG4:CO_RG4 + 4] = (k[:16, None] // 4 == np.arange(4)[None, :])
    ct[:64, CO_ONES:CO_ONES + 64] = 1.0
    ct[:64, CO_QIOTA] = k[:64]
    ct[0, CO_PIOTA:CO_PIOTA + 64] = np.arange(64)
    ct[:64, CO_WSEL:CO_WSEL + 16] = (k[:64, None] % 16 == np.arange(16)[None, :])
    ct[:4, CO_ID4:CO_ID4 + 4] = np.eye(4)
    ct[0, CO_ID1] = 1.0
    ct[:, CO_CGRP:CO_CGRP + 4] = (k[:, None] // 32 == np.arange(4)[None, :])
    ct[:4, CO_BLK:CO_BLK + 256] = (k[:4, None] == np.arange(256)[None, :] // 64)
    ct[:64, CO_WM4:CO_WM4 + 16] = (k[:64, None] // 16 == np.arange(16)[None, :] % 4)
    ct[:, CO_SMASK:CO_SMASK + 16] = (k[:, None] // 32 == np.arange(16)[None, :] // 4)
    ct[:16, CO_PISEL:CO_PISEL + 128] = (k[:16, None] == np.arange(128)[None, :] % 16)
    return ct


def _bcast_free(ap: bass.AP, dims) -> bass.AP:
    """Insert stride-0 free dims at the given (pos, count) positions."""
    new = [list(p) for p in ap.ap]
    for pos, count in dims:
        new.insert(pos, [0, count])
    return bass.AP(ap.tensor, ap.offset, new)


def build_core_kernel(debug: bool = False, dbg_out: bool = False) -> bacc.Bacc:
    nc = bacc.Bacc("TRN2", target_bir_lowering=False, debug=debug,
                   num_devices=N_CORES)

    pred_d = nc.dram_tensor("pred", [SPB, NCLS, IMG, IMG], F32, kind="ExternalInput")
    oimg_d = nc.dram_tensor("oimg", [SPB, 3, IMG, IMG], F32, kind="ExternalInput")
    aimg_d = nc.dram_tensor("aimg", [SPB, 3, IMG, IMG], F32, kind="ExternalInput")
    olab_d = nc.dram_tensor("olab", [SPB, IMG, IMG], I32, kind="ExternalInput")
    alab_d = nc.dram_tensor("alab", [SPB, IMG, IMG], I32, kind="ExternalInput")
    ocnf_d = nc.dram_tensor("ocnf", [SPB, IMG, IMG], F32, kind="ExternalInput")
    acnf_d = nc.dram_tensor("acnf", [SPB, IMG, IMG], F32, kind="ExternalInput")
    cst_d = nc.dram_tensor("consts", [128, CW], F32, kind="ExternalInput")

    if dbg_out:
        dbg_d = {
            "dbg_loss": nc.dram_tensor("dbg_loss", [NG, 1, 4], F32, kind="ExternalOutput"),
            "dbg_m": nc.dram_tensor("dbg_m", [NG, 1, 4], F32, kind="ExternalOutput"),
            "dbg_sg": nc.dram_tensor("dbg_sg", [NG, 1, 4], F32, kind="ExternalOutput"),
            "dbg_rp1": nc.dram_tensor("dbg_rp1", [NG, 1, 256], F32, kind="ExternalOutput"),
            "dbg_repl": nc.dram_tensor("dbg_repl", [NG, 1, 256], F32, kind="ExternalOutput"),
            "dbg_sidx": nc.dram_tensor("dbg_sidx", [NG, 1, 256], F32, kind="ExternalOutput"),
            "dbg_racs": nc.dram_tensor("dbg_racs", [NG, 64, 4], F32, kind="ExternalOutput"),
            "dbg_idxf": nc.dram_tensor("dbg_idxf", [NG, 128, 4], F32, kind="ExternalOutput"),
        }
    out_img = nc.dram_tensor("out_img", [SPB, 3, IMG, IMG], F32, kind="ExternalOutput")
    out_lab = nc.dram_tensor("out_lab", [SPB, IMG, IMG], I32, kind="ExternalOutput")
    out_cnf = nc.dram_tensor("out_cnf", [SPB, IMG, IMG], F32, kind="ExternalOutput")

    def patch_view(ap):
        # [ns, 256, 256] -> [ns, 32, 8, 256]  (r, bi, (bj w)) per sample
        return ap.rearrange("s (bi r) (bj w) -> s r bi (bj w)", bi=NH, bj=NH)

    with tile.TileContext(nc) as tc:
        with (
            tc.tile_pool(name="cstp", bufs=1) as cstp,
            tc.tile_pool(name="predp", bufs=1) as predp,
            tc.tile_pool(name="dice", bufs=2) as dice,
            tc.tile_pool(name="dice_e", bufs=2) as dice_e,
            tc.tile_pool(name="lnp", bufs=4) as lnp,
            tc.tile_pool(name="parts", bufs=3 * NG) as parts,
            tc.tile_pool(name="keys", bufs=2) as keys,
            tc.tile_pool(name="tiny", bufs=8) as tiny,
            tc.tile_pool(name="chin", bufs=5) as chin,
            tc.tile_pool(name="chout", bufs=3) as chout,
            tc.tile_pool(name="ps512", bufs=4, space="PSUM") as ps512,
            tc.tile_pool(name="ps256", bufs=2, space="PSUM") as ps256,
            tc.tile_pool(name="pstiny", bufs=2, space="PSUM") as pstiny,
        ):
            ct = cstp.tile([128, CW], F32, tag="ct")
            nc.sync.dma_start(ct, cst_d[:])
            cls_ones4 = ct[:, CO_BC32:CO_BC32 + 128]
            c_col = ct[:, CO_CCOL:CO_CCOL + 1]
            rg8 = ct[:, CO_RG8:CO_RG8 + 16]
            rg4 = ct[0:16, CO_RG4:CO_RG4 + 4]
            ones64 = ct[0:64, CO_ONES:CO_ONES + 64]
            qiota = ct[0:64, CO_QIOTA:CO_QIOTA + 1]
            piota = ct[0:1, CO_PIOTA:CO_PIOTA + 64]
            wsel = ct[0:64, CO_WSEL:CO_WSEL + 16]
            wmask4 = ct[0:64, CO_WM4:CO_WM4 + 16]
            smask = ct[:, CO_SMASK:CO_SMASK + 16]
            pisel = ct[0:16, CO_PISEL:CO_PISEL + 128]
            id4 = ct[0:4, CO_ID4:CO_ID4 + 4]
            id1 = ct[0:1, CO_ID1:CO_ID1 + 1]
            cls_grp = ct[:, CO_CGRP:CO_CGRP + 4]
            blkmask = ct[0:4, CO_BLK:CO_BLK + 256]
            bc32 = ct[0:64, CO_BC32:CO_BC32 + 128]
            # bf16 copy for the label-broadcast matmuls (labels exact 0..3)
            bc32h = cstp.tile([64, 128], BF16, tag="bc32h")
            nc.vector.tensor_copy(bc32h, bc32)

            idxts = []
            cfins = []
            for g in range(NG):
                s0 = g * GS

                # ---------------- dice loss ----------------
                scp = parts.tile([128, 16], F32, tag="part")
                intp = parts.tile([128, 16], F32, tag="part")
                cntp = parts.tile([128, 16], F32, tag="part")
                labv = olab_d[:].rearrange("s (a b) w -> s a (b w)", a=32)

                for hp in range(2):
                    e_ts = []
                    for sj in range(2):
                        si = 2 * hp + sj
                        pt = predp.tile([128, 2048], F32, tag="pred")
                        nc.sync.dma_start(
                            pt, pred_d[s0 + si].rearrange(
                                "c (a b) w -> (c a) (b w)", a=32))
                        et = dice_e.tile([128, 2048], F32, tag="e")
                        nc.scalar.activation(et, pt, AF.Exp)
                        e_ts.append(et)
                    # labels pair tile as bf16 [64=(s2,ph), 2048]
                    lf = dice.tile([64, 2048], BF16, tag="labf")
                    nc.gpsimd.dma_start(
                        out=lf, in_=labv[s0 + 2 * hp:s0 + 2 * hp + 2])

                    for sj in range(2):
                        si = 2 * hp + sj
                        psl = slice(32 * sj, 32 * sj + 32)
                        # r = exp(-ln(S)) replicated over class groups;
                        # batch Ln then Exp to avoid ACT table reloads
                        rrep = dice.tile([128, 2048], F32, tag="rrep")
                        spss = []
                        for ch in range(4):
                            cs = slice(ch * 512, ch * 512 + 512)
                            sps = ps512.tile([128, 512], F32, space="PSUM",
                                             tag="pb")
                            nc.tensor.matmul(sps, lhsT=cls_ones4,
                                             rhs=e_ts[sj][:, cs],
                                             start=True, stop=True)
                            spss.append(sps)
                        lnts = []
                        for ch in range(4):
                            lnt = lnp.tile([128, 512], F32, tag="lnS")
                            nc.scalar.activation(lnt, spss[ch], AF.Ln)
                            lnts.append(lnt)
                        for ch in range(4):
                            cs = slice(ch * 512, ch * 512 + 512)
                            nc.scalar.activation(rrep[:, cs], lnts[ch], AF.Exp,
                                                 scale=-1.0)
                        # s_c = e * r ; accum -> sum_s partials (per chunk)
                        scx = dice.tile([128, 2048], F32, tag="scx")
                        for ch in range(4):
                            cs = slice(ch * 512, ch * 512 + 512)
                            col = slice(4 * si + ch, 4 * si + ch + 1)
                            nc.vector.affine_mul_reduce(
                                scx[:, cs], scp[:, col],
                                e_ts[sj][:, cs], rrep[:, cs], 1.0, 0.0)
                        for ch in range(4):
                            cs = slice(ch * 512, ch * 512 + 512)
                            col = slice(4 * si + ch, 4 * si + ch + 1)
                            lbp = ps512.tile([128, 512], F32, space="PSUM",
                                             tag="pb")
                            nc.tensor.matmul(lbp, lhsT=bc32h[psl, :],
                                             rhs=lf[psl, cs],
                                             start=True, stop=True)
                            # count: mask with accum (junk out tile)
                            mjk = dice.tile([128, 512], F32, tag="mjk")
                            nc.any.tensor_scalar(mjk, lbp, c_col, None,
                                                 AL.is_equal, AL.add,
                                                 accum_out=cntp[:, col])
                            # inter = mask * s_c ; accum -> inter partial
                            nc.vector.scalar_tensor_tensor(
                                scx[:, cs], lbp, c_col, scx[:, cs],
                                AL.is_equal, AL.mult, accum_out=intp[:, col])

                # reduce partials -> per (class, sample) -> loss row
                q3 = []
                for ptile in (scp, intp, cntp):
                    pq = pstiny.tile([4, 16], F32, space="PSUM", tag="tp")
                    nc.tensor.matmul(pq, lhsT=cls_grp, rhs=ptile,
                                     start=True, stop=True)
                    q = tiny.tile([4, 4], F32, tag="q44")
                    nc.vector.tensor_reduce(q, pq.rearrange("c (s h) -> c s h", s=4),
                                            axis=mybir.AxisListType.X, op=AL.add)
                    q3.append(q)
                ss4, int4, cnt4 = q3
                u = tiny.tile([4, 4], F32, tag="q44")
                nc.vector.scalar_tensor_tensor(u, ss4, 1e-5, cnt4, AL.add, AL.add)
                iu = tiny.tile([4, 4], F32, tag="q44")
                nc.vector.reciprocal(iu, u)
                d4 = tiny.tile([4, 4], F32, tag="q44")
                nc.vector.scalar_tensor_tensor(d4, int4, 2.0, iu, AL.mult, AL.mult)
                lsump = pstiny.tile([1, 4], F32, space="PSUM", tag="tp")
                nc.tensor.matmul(lsump, lhsT=ones64[0:4, 0:1], rhs=d4,
                                 start=True, stop=True)
                loss = tiny.tile([1, 4], F32, tag="row4")
                nc.vector.tensor_scalar(loss, lsump, -1.0 / NCLS, 1.0,
                                        AL.mult, AL.add)
                trow = tiny.tile([1, 4], F32, tag="row4")
                nc.vector.tensor_scalar(trow, loss, -16.0 / AGE_DIV, 16.0,
                                        AL.mult, AL.add)
                tneg = tiny.tile([1, 4], F32, tag="row4")
                nc.vector.tensor_scalar(tneg, trow, -1.0, None, AL.mult)
                ta = tiny.tile([1, 4], F32, tag="row4")
                nc.vector.tensor_tensor(ta, trow, tneg, op=AL.max)
                mrow = tiny.tile([1, 4], F32, tag="row4")
                nc.vector.tensor_scalar(mrow, ta, 16.0, None, AL.min)
                # sg01 = [loss < 1]  (1 when the self-paced branch flips sort)
                sgrow = tiny.tile([1, 4], F32, tag="row4")
                nc.vector.tensor_scalar(sgrow, loss, 1.0, None, AL.is_lt)

                # -------- keys: conf patch sums + sign-free ranks --------
                cfin = chin.tile([128, 128 * 32], F32, tag="cin")
                for si in range(GS):
                    row = slice(32 * si, 32 * si + 32)
                    nc.sync.dma_start(cfin[row, 0:2048],
                                      patch_view(ocnf_d[:])[s0 + si])
                    nc.sync.dma_start(cfin[row, 2048:4096],
                                      patch_view(acnf_d[:])[s0 + si])
                cfins.append(cfin)
                colred = keys.tile([128, 128], F32, tag="colred")
                nc.vector.tensor_reduce(colred,
                                        cfin[:].rearrange("p (n d) -> p n d", d=32),
                                        axis=mybir.AxisListType.X, op=AL.add)
                st1p = pstiny.tile([16, 128], F32, space="PSUM", tag="tp")
                nc.tensor.matmul(st1p, lhsT=rg8, rhs=colred, start=True, stop=True)
                st1 = keys.tile([16, 128], F32, tag="st1")
                nc.vector.tensor_copy(st1, st1p)
                ksump = pstiny.tile([4, 128], F32, space="PSUM", tag="tp")
                nc.tensor.matmul(ksump, lhsT=rg4, rhs=st1, start=True, stop=True)
                kT = keys.tile([4, 128], F32, tag="kT")
                nc.vector.tensor_copy(kT, ksump)
                kcp = pstiny.tile([128, 4], F32, space="PSUM", tag="tp")
                nc.tensor.transpose(kcp, kT, id4)
                kcols = keys.tile([128, 4], F32, tag="kcols")
                nc.vector.tensor_copy(kcols, kcp)

                # block-diagonal expansion: rhs[k,(si,p)] = kT[k,p]*[si==k]
                blk3 = blkmask.rearrange("k (s p) -> k s p", s=4)
                kTeo = keys.tile([4, 256], F32, tag="kTe")
                nc.vector.tensor_tensor(
                    kTeo[:].rearrange("k (s p) -> k s p", s=4),
                    _bcast_free(kT[0:4, 0:64], [(1, 4)]), blk3, op=AL.mult)
                kTea = keys.tile([4, 256], F32, tag="kTe")
                nc.vector.tensor_tensor(
                    kTea[:].rearrange("k (s p) -> k s p", s=4),
                    _bcast_free(kT[0:4, 64:128], [(1, 4)]), blk3, op=AL.mult)
                obc = ps256.tile([64, 256], F32, space="PSUM", tag="k2")
                abc = ps256.tile([64, 256], F32, space="PSUM", tag="k2")
                nc.tensor.matmul(obc, lhsT=ones64[0:4, :], rhs=kTeo,
                                 start=True, stop=True)
                nc.tensor.matmul(abc, lhsT=ones64[0:4, :], rhs=kTea,
                                 start=True, stop=True)
                # A+[q,(s,p)] = [raw_key_s[p] > raw_key_s[q]]
                Ao = keys.tile([64, 256], F32, tag="Ao")
                nc.vector.tensor_tensor(
                    Ao, obc, _bcast_free(kcols[0:64, :], [(2, 64)]), op=AL.is_gt)
                Aa = keys.tile([64, 256], F32, tag="Aa")
                nc.vector.tensor_tensor(
                    Aa, abc, _bcast_free(kcols[64:128, :], [(2, 64)]), op=AL.is_gt)
                # ascending ranks of raw sums
                rorp = ps256.tile([64, 256], F32, space="PSUM", tag="k2")
                racp = pstiny.tile([64, 4], F32, space="PSUM", tag="tp")
                for si in range(GS):
                    osl = slice(64 * si, 64 * si + 64)
                    nc.tensor.matmul(rorp[:, osl], lhsT=ones64, rhs=Ao[:, osl],
                                     start=True, stop=True)
                    nc.tensor.matmul(racp[:, si:si + 1], lhsT=Aa[:, osl],
                                     rhs=ones64[:, 0:1], start=True, stop=True)
                rors = keys.tile([64, 256], F32, tag="rors")
                nc.vector.tensor_copy(rors, rorp)
                racs = keys.tile([64, 4], F32, tag="racs")
                nc.vector.tensor_copy(racs, racp)

                # -------- fixup with sign + build gather index --------
                # sg broadcast [64, (s)] and [64, (s, p)]
                sgp = pstiny.tile([64, 4], F32, space="PSUM", tag="tp")
                nc.tensor.matmul(sgp, lhsT=ones64[0:1, :], rhs=sgrow,
                                 start=True, stop=True)
                sgb = keys.tile([64, 4], F32, tag="sgb")
                nc.vector.tensor_copy(sgb, sgp)
                # rank_o = r+ + sg*(63 - 2 r+)   (descending when sg=1)
                rob = keys.tile([64, 256], F32, tag="rob")
                nc.vector.tensor_scalar(rob, rors, -2.0, 63.0, AL.mult, AL.add)
                nc.vector.tensor_tensor(
                    rob, rob, _bcast_free(sgb[:, :], [(2, 64)]), op=AL.mult)
                nc.vector.tensor_tensor(rob, rob, rors, op=AL.add)
                # rank_a = r_a+ + (1-sg)*(63 - 2 r_a+)
                rac2 = keys.tile([64, 4], F32, tag="rac2")
                nc.vector.tensor_scalar(rac2, racs, -2.0, 63.0, AL.mult, AL.add)
                sga = keys.tile([64, 4], F32, tag="sga")
                nc.vector.tensor_scalar(sga, sgb, -1.0, 1.0, AL.mult, AL.add)
                nc.vector.tensor_tensor(rac2, rac2, sga, op=AL.mult)
                nc.vector.tensor_tensor(rac2, rac2, racs, op=AL.add)

                # M2[q,(s,p)] = [rank_a[q] == rank_o[p]] ; src = q @ M2
                M2 = keys.tile([64, 256], F32, tag="M2")
                nc.vector.tensor_tensor(
                    M2, rob, _bcast_free(rac2[:, :], [(2, 64)]), op=AL.is_equal)
                srcp = pstiny.tile([1, 256], F32, space="PSUM", tag="tp")
                nc.tensor.matmul(srcp, lhsT=qiota, rhs=M2, start=True, stop=True)

                rp1 = keys.tile([1, 256], F32, tag="rowt")
                nc.vector.tensor_scalar(rp1, rob[0:1, :], 1.0, None, AL.add)
                repl = keys.tile([1, 256], F32, tag="rowt")
                nc.vector.tensor_tensor(
                    repl, rp1, _bcast_free(mrow[0:1, :], [(2, 64)]), op=AL.is_le)
                x1 = keys.tile([1, 256], F32, tag="rowt")
                pio = _bcast_free(piota, [(1, 4)])
                nc.vector.tensor_tensor(x1, srcp, pio, op=AL.subtract)
                nc.vector.tensor_scalar(x1, x1, 64.0, None, AL.add)
                nc.vector.tensor_tensor(x1, x1, repl, op=AL.mult)
                nc.vector.tensor_tensor(x1, x1, pio, op=AL.add)

                sicp = pstiny.tile([64, 4], F32, space="PSUM", tag="tp")
                for si in range(GS):
                    nc.tensor.transpose(sicp[:, si:si + 1],
                                        x1[0:1, 64 * si:64 * si + 64], id1)
                sics = keys.tile([64, 4], F32, tag="sics")
                nc.vector.tensor_copy(sics, sicp)
                # w1g[p, 4s+phi] = sidx_s[p] * [p//16 == phi]
                w1g = keys.tile([64, 16], F32, tag="w1g")
                nc.vector.tensor_tensor(
                    w1g[:].rearrange("p (s f) -> p s f", s=4),
                    _bcast_free(sics[:, :], [(2, 4)]),
                    wmask4.rearrange("p (s f) -> p s f", s=4), op=AL.mult)
                # wig[pi, 4s+phi] = sidx_s[16 phi + pi]
                wig = pstiny.tile([16, 16], F32, space="PSUM", tag="tp")
                nc.tensor.matmul(wig, lhsT=wsel, rhs=w1g, start=True, stop=True)
                wigs = keys.tile([16, 16], F32, tag="wigs")
                nc.vector.tensor_copy(wigs, wig)
                # replicate rows to all 128 partitions, select own sample block
                wbig = ps256.tile([128, 16], F32, space="PSUM", tag="k2")
                nc.tensor.matmul(wbig, lhsT=pisel, rhs=wigs, start=True, stop=True)
                wsl = keys.tile([128, 16], F32, tag="wsl")
                nc.vector.tensor_tensor(wsl, wbig, smask, op=AL.mult)
                idxf = keys.tile([128, 4], F32, tag="idxf")
                ws_v = bass.AP(wsl[:].tensor, wsl[:].offset,
                               [list(wsl[:].ap[0]), [1, 4], [4, 4]])
                nc.vector.tensor_reduce(idxf, ws_v,
                                        axis=mybir.AxisListType.X, op=AL.add)
                idxt = keys.tile([128, 4], I16, tag="idxt")
                nc.vector.tensor_copy(idxt, idxf)
                idxts.append(idxt)

                if dbg_out:
                    for nm, t in [("dbg_loss", loss), ("dbg_m", mrow),
                                  ("dbg_sg", sgrow), ("dbg_rp1", rp1),
                                  ("dbg_repl", repl), ("dbg_sidx", x1),
                                  ("dbg_racs", rac2), ("dbg_idxf", idxf)]:
                        nc.sync.dma_start(dbg_d[nm][g], t)

            # ---------------- gather + write back, all chunks ----------------
            def load_chunk(g, osrcv, asrcv, dt):
                s0 = g * GS
                cin = chin.tile([128, 128 * 32], dt, tag="cin")
                for si in range(GS):
                    row = slice(32 * si, 32 * si + 32)
                    nc.sync.dma_start(cin[row, 0:2048], osrcv[s0 + si])
                    nc.sync.dma_start(cin[row, 2048:4096], asrcv[s0 + si])
                return cin

            def gather_store(g, cin, dstv, dt):
                s0 = g * GS
                cout = chout.tile([128, 64 * 32], dt, tag="cout")
                nc.gpsimd.ap_gather(
                    cout[:].rearrange("p (n d) -> p n d", d=32),
                    cin[:].rearrange("p (n d) -> p n d", d=32),
                    idxts[g][:],
                    channels=128, num_elems=128, d=32, num_idxs=64)
                for si in range(GS):
                    nc.sync.dma_start(
                        dstv[s0 + si],
                        cout[32 * si:32 * si + 32, :].rearrange(
                            "r (bi f) -> r bi f", bi=NH))

            chunks = [
                (patch_view(oimg_d[:, 0]), patch_view(aimg_d[:, 0]),
                 patch_view(out_img[:, 0]), F32),
                (patch_view(oimg_d[:, 1]), patch_view(aimg_d[:, 1]),
                 patch_view(out_img[:, 1]), F32),
                (patch_view(oimg_d[:, 2]), patch_view(aimg_d[:, 2]),
                 patch_view(out_img[:, 2]), F32),
                (patch_view(olab_d[:]), patch_view(alab_d[:]),
                 patch_view(out_lab[:]), I32),
            ]
            # stream: conf gathers first (tiles already resident), then the
            # 4 remaining chunks per group, loads running ahead of gathers
            pend = []
            for g in range(NG):
                for (ov, av, dv, dt) in chunks:
                    pend.append((g, load_chunk(g, ov, av, dt), dv, dt))
            for g in range(NG):
                gather_store(g, cfins[g], patch_view(out_cnf[:]), F32)
            for (g, cin, dv, dt) in pend:
                gather_store(g, cin, dv, dt)

    nc.compile()
    return nc


_cached = {}


def kernel(oimage, aimage, olabel, alabel, oconf, aconf, prediction,
           cur_step=None):
    if "nc" not in _cached:
        _cached["nc"] = build_core_kernel(debug=False)
    nc = _cached["nc"]
    cst = _build_consts()
    in_maps = []
    for i in range(N_CORES):
        sl = slice(i * SPB, (i + 1) * SPB)
        in_maps.append({
            "pred": np.ascontiguousarray(prediction[sl], np.float32),
            "oimg": np.ascontiguousarray(oimage[sl], np.float32),
            "aimg": np.ascontiguousarray(aimage[sl], np.float32),
            "olab": np.ascontiguousarray(olabel[sl], np.int32),
            "alab": np.ascontiguousarray(alabel[sl], np.int32),
            "ocnf": np.ascontiguousarray(oconf[sl], np.float32),
            "acnf": np.ascontiguousarray(aconf[sl], np.float32),
            "consts": cst,
        })
    res = run_bass_kernel_spmd(nc, in_maps, core_ids=list(range(N_CORES)))
    _exec_info["exec_time_ns"] = res.exec_time_ns
    img = np.concatenate([res.results[i]["out_img"] for i in range(N_CORES)])
    lab = np.concatenate([res.results[i]["out_lab"] for i in range(N_CORES)])
    cnf = np.concatenate([res.results[i]["out_cnf"] for i in range(N_CORES)])
    return img, lab.astype(np.int32), cnf


# revision 53
# speedup vs baseline: 1.9056x; 1.0836x over previous
"""AdaMix forward on 8 Trainium2 NeuronCores (Bass/Tile), pure data parallel.

Per core: 8 samples, processed as 2 groups of 4. Phases (heavily pipelined by
Tile): per group -> dice loss (classes-on-partitions, replicated class-sum
matmul, r = exp(-ln(S)) on ScalarE, fused AMR/STT accumulation), conf patch
sums + sign-independent ranks (all-pairs compares + one-hot matmuls), tiny
sign/k fixup -> per-sample gather index; then all channel chunks stream
through gpsimd ap_gather (o patches = slots 0..63, a = 64..127).
"""

import os
import numpy as np

import concourse.bass as bass
import concourse.mybir as mybir
import concourse.tile as tile
from concourse import bacc
from concourse.bass_utils import run_bass_kernel_spmd

F32 = mybir.dt.float32
BF16 = mybir.dt.bfloat16
I32 = mybir.dt.int32
I16 = mybir.dt.int16
AL = mybir.AluOpType
AF = mybir.ActivationFunctionType

N_CORES = 8
B = 64
SPB = B // N_CORES      # samples per core
GS = 4                  # samples per group
NG = SPB // GS          # groups per core
NCLS = 4
IMG = 256
PS = 32                 # patch side
NH = 8                  # patches per image side
NP = 64                 # patches per image
AGE_DIV = 1.0 + 1e-5

# ---------------- constant pack (one [128, CW] f32 tensor) ----------------
CO_CLS = 0      # [128,32]  cls_ones:  k%32==m   (class sum over c)
CO_BC32 = 32    # [128,128] bc32:      m%32==k%32 (replicated broadcast)
CO_CCOL = 160   # [128,1]   c_col:     k//32
CO_RG8 = 161    # [128,16]  rgrp8:     k//8==m
CO_RG4 = 177    # [16,4]    rgrp4:     k//4==m
CO_ONES = 181   # [64,64]   ones
CO_QIOTA = 245  # [64,1]    qiota:     k
CO_PIOTA = 246  # [1,64]    piota:     m
CO_WMASK = 310  # [64,4]    (unused)
CO_WSEL = 314   # [64,16]   wrapsel:   k%16==m
CO_ID4 = 330    # [4,4]     identity
CO_ID1 = 334    # [1,1]     identity
CO_CGRP = 335   # [128,4]   cls_grp:   k//32==m
CO_BLK = 339    # [4,256]   blkmask:   k==m//64
CO_WM4 = 595    # [64,16]   wmask4:    k//16==m%4
CO_SMASK = 611  # [128,16]  smask:     k//32==m//4
CO_PISEL = 627  # [16,128]  pisel:     k==m%16
CW = 756

_exec_info = {}


def _install_ntff_hook():
    """The agent image's antenv lacks axon_hooks; rebuild it from the boot
    helpers so trace=True (BASS_TRACE=1) works for profiling."""
    import sys
    import types
    try:
        import antenv.axon_hooks  # noqa: F401
        return
    except ImportError:
        pass
    try:
        import antenv
        from trn_agent_boot.trn_boot import _ntff_profile_via_ctypes
        hook = _ntff_profile_via_ctypes("/opt/axon/libaxon_pjrt.so")
        mod = types.ModuleType("antenv.axon_hooks")
        state = {"hook": hook}
        mod.set_axon_ntff_profile_hook = lambda h: state.update(hook=h)
        mod.get_axon_ntff_profile_hook = lambda: state["hook"]
        sys.modules["antenv.axon_hooks"] = mod
        antenv.axon_hooks = mod
    except Exception:
        pass


_install_ntff_hook()


def _build_consts() -> np.ndarray:
    ct = np.zeros((128, CW), np.float32)
    k = np.arange(128)
    ct[:, CO_CLS:CO_CLS + 32] = (k[:, None] % 32 == np.arange(32)[None, :])
    ct[:, CO_BC32:CO_BC32 + 128] = (np.arange(128)[None, :] % 32 == k[:, None] % 32)
    ct[:, CO_CCOL] = k // 32
    ct[:, CO_RG8:CO_RG8 + 16] = (k[:, None] // 8 == np.arange(16)[None, :])
    ct[:16, CO_RG4:CO_RG4 + 4] = (k[:16, None] // 4 == np.arange(4)[None, :])
    ct[:64, CO_ONES:CO_ONES + 64] = 1.0
    ct[:64, CO_QIOTA] = k[:64]
    ct[0, CO_PIOTA:CO_PIOTA + 64] = np.arange(64)
    ct[:64, CO_WSEL:CO_WSEL + 16] = (k[:64, None] % 16 == np.arange(16)[None, :])
    ct[:4, CO_ID4:CO_ID4 + 4] = np.eye(4)
    ct[0, CO_ID1] = 1.0
    ct[:, CO_CGRP:CO_CGRP + 4] = (k[:, None] // 32 == np.arange(4)[None, :])
    ct[:4, CO_BLK:CO_BLK + 256] = (k[:4, None] == np.arange(256)[None, :] // 64)
    ct[:64, CO_WM4:CO_WM4 + 16] = (k[:64, None] // 16 == np.arange(16)[None, :] % 4)
    ct[:, CO_SMASK:CO_SMASK + 16] = (k[:, None] // 32 == np.arange(16)[None, :] // 4)
    ct[:16, CO_PISEL:CO_PISEL + 128] = (k[:16, None] == np.arange(128)[None, :] % 16)
    return ct


def _bcast_free(ap: bass.AP, dims) -> bass.AP:
    """Insert stride-0 free dims at the given (pos, count) positions."""
    new = [list(p) for p in ap.ap]
    for pos, count in dims:
        new.insert(pos, [0, count])
    return bass.AP(ap.tensor, ap.offset, new)


def build_core_kernel(debug: bool = False, dbg_out: bool = False) -> bacc.Bacc:
    nc = bacc.Bacc("TRN2", target_bir_lowering=False, debug=debug,
                   num_devices=N_CORES)

    pred_d = nc.dram_tensor("pred", [SPB, NCLS, IMG, IMG], F32, kind="ExternalInput")
    oimg_d = nc.dram_tensor("oimg", [SPB, 3, IMG, IMG], F32, kind="ExternalInput")
    aimg_d = nc.dram_tensor("aimg", [SPB, 3, IMG, IMG], F32, kind="ExternalInput")
    olab_d = nc.dram_tensor("olab", [SPB, IMG, IMG], I32, kind="ExternalInput")
    alab_d = nc.dram_tensor("alab", [SPB, IMG, IMG], I32, kind="ExternalInput")
    ocnf_d = nc.dram_tensor("ocnf", [SPB, IMG, IMG], F32, kind="ExternalInput")
    acnf_d = nc.dram_tensor("acnf", [SPB, IMG, IMG], F32, kind="ExternalInput")
    cst_d = nc.dram_tensor("consts", [128, CW], F32, kind="ExternalInput")

    if dbg_out:
        dbg_d = {
            "dbg_loss": nc.dram_tensor("dbg_loss", [NG, 1, 4], F32, kind="ExternalOutput"),
            "dbg_m": nc.dram_tensor("dbg_m", [NG, 1, 4], F32, kind="ExternalOutput"),
            "dbg_sg": nc.dram_tensor("dbg_sg", [NG, 1, 4], F32, kind="ExternalOutput"),
            "dbg_rp1": nc.dram_tensor("dbg_rp1", [NG, 1, 256], F32, kind="ExternalOutput"),
            "dbg_repl": nc.dram_tensor("dbg_repl", [NG, 1, 256], F32, kind="ExternalOutput"),
            "dbg_sidx": nc.dram_tensor("dbg_sidx", [NG, 1, 256], F32, kind="ExternalOutput"),
            "dbg_racs": nc.dram_tensor("dbg_racs", [NG, 64, 4], F32, kind="ExternalOutput"),
            "dbg_idxf": nc.dram_tensor("dbg_idxf", [NG, 128, 4], F32, kind="ExternalOutput"),
        }
    out_img = nc.dram_tensor("out_img", [SPB, 3, IMG, IMG], F32, kind="ExternalOutput")
    out_lab = nc.dram_tensor("out_lab", [SPB, IMG, IMG], I32, kind="ExternalOutput")
    out_cnf = nc.dram_tensor("out_cnf", [SPB, IMG, IMG], F32, kind="ExternalOutput")

    def patch_view(ap):
        # [ns, 256, 256] -> [ns, 32, 8, 256]  (r, bi, (bj w)) per sample
        return ap.rearrange("s (bi r) (bj w) -> s r bi (bj w)", bi=NH, bj=NH)

    with tile.TileContext(nc) as tc:
        with (
            tc.tile_pool(name="cstp", bufs=1) as cstp,
            tc.tile_pool(name="predp", bufs=2) as predp,
            tc.tile_pool(name="dice", bufs=2) as dice,
            tc.tile_pool(name="dice_e", bufs=2) as dice_e,

            tc.tile_pool(name="parts", bufs=3 * NG) as parts,
            tc.tile_pool(name="keys", bufs=2) as keys,
            tc.tile_pool(name="tiny", bufs=8) as tiny,
            tc.tile_pool(name="chin", bufs=5) as chin,
            tc.tile_pool(name="chout", bufs=3) as chout,
            tc.tile_pool(name="ps512", bufs=4, space="PSUM") as ps512,
            tc.tile_pool(name="ps256", bufs=2, space="PSUM") as ps256,
            tc.tile_pool(name="pstiny", bufs=2, space="PSUM") as pstiny,
        ):
            ct = cstp.tile([128, CW], F32, tag="ct")
            nc.sync.dma_start(ct, cst_d[:])
            cls_ones4 = ct[:, CO_BC32:CO_BC32 + 128]
            c_col = ct[:, CO_CCOL:CO_CCOL + 1]
            rg8 = ct[:, CO_RG8:CO_RG8 + 16]
            rg4 = ct[0:16, CO_RG4:CO_RG4 + 4]
            ones64 = ct[0:64, CO_ONES:CO_ONES + 64]
            qiota = ct[0:64, CO_QIOTA:CO_QIOTA + 1]
            piota = ct[0:1, CO_PIOTA:CO_PIOTA + 64]
            wsel = ct[0:64, CO_WSEL:CO_WSEL + 16]
            wmask4 = ct[0:64, CO_WM4:CO_WM4 + 16]
            smask = ct[:, CO_SMASK:CO_SMASK + 16]
            pisel = ct[0:16, CO_PISEL:CO_PISEL + 128]
            id4 = ct[0:4, CO_ID4:CO_ID4 + 4]
            id1 = ct[0:1, CO_ID1:CO_ID1 + 1]
            cls_grp = ct[:, CO_CGRP:CO_CGRP + 4]
            blkmask = ct[0:4, CO_BLK:CO_BLK + 256]
            bc32 = ct[0:64, CO_BC32:CO_BC32 + 128]
            # bf16 copy for the label-broadcast matmuls (labels exact 0..3)
            bc32h = cstp.tile([64, 128], BF16, tag="bc32h")
            nc.vector.tensor_copy(bc32h, bc32)

            idxts = []
            cfins = []
            for g in range(NG):
                s0 = g * GS

                # ---------------- dice loss ----------------
                scp = parts.tile([128, 16], F32, tag="part")
                intp = parts.tile([128, 16], F32, tag="part")
                cntp = parts.tile([128, 16], F32, tag="part")
                labv = olab_d[:].rearrange("s (a b) w -> s a (b w)", a=32)

                for hp in range(2):
                    e_ts = []
                    for sj in range(2):
                        si = 2 * hp + sj
                        pt = predp.tile([128, 2048], F32, tag="pred")
                        nc.scalar.dma_start(
                            pt, pred_d[s0 + si].rearrange(
                                "c (a b) w -> (c a) (b w)", a=32))
                        et = dice_e.tile([128, 2048], F32, tag="e")
                        nc.scalar.activation(et, pt, AF.Exp)
                        e_ts.append(et)
                    # labels pair tile as bf16 [64=(s2,ph), 2048]
                    lf = dice.tile([64, 2048], BF16, tag="labf")
                    nc.gpsimd.dma_start(
                        out=lf, in_=labv[s0 + 2 * hp:s0 + 2 * hp + 2])

                    for sj in range(2):
                        si = 2 * hp + sj
                        psl = slice(32 * sj, 32 * sj + 32)
                        # r = exp(-ln(S)) replicated over class groups;
                        # batch Ln then Exp to avoid ACT table reloads
                        rrep = dice.tile([128, 2048], F32, tag="rrep")
                        spss = []
                        for ch in range(4):
                            cs = slice(ch * 512, ch * 512 + 512)
                            sps = ps512.tile([128, 512], F32, space="PSUM",
                                             tag="pb")
                            nc.tensor.matmul(sps, lhsT=cls_ones4,
                                             rhs=e_ts[sj][:, cs],
                                             start=True, stop=True)
                            spss.append(sps)
                        for ch in range(4):
                            cs = slice(ch * 512, ch * 512 + 512)
                            nc.scalar.activation(rrep[:, cs], spss[ch], AF.Ln)
                        for ch in range(4):
                            cs = slice(ch * 512, ch * 512 + 512)
                            nc.scalar.activation(rrep[:, cs], rrep[:, cs],
                                                 AF.Exp, scale=-1.0)
                        # s_c = e * r ; accum -> sum_s partials (per chunk)
                        scx = dice.tile([128, 2048], F32, tag="scx")
                        for ch in range(4):
                            cs = slice(ch * 512, ch * 512 + 512)
                            col = slice(4 * si + ch, 4 * si + ch + 1)
                            nc.vector.affine_mul_reduce(
                                scx[:, cs], scp[:, col],
                                e_ts[sj][:, cs], rrep[:, cs], 1.0, 0.0)
                        for ch in range(4):
                            cs = slice(ch * 512, ch * 512 + 512)
                            col = slice(4 * si + ch, 4 * si + ch + 1)
                            lbp = ps512.tile([128, 512], F32, space="PSUM",
                                             tag="pb")
                            nc.tensor.matmul(lbp, lhsT=bc32h[psl, :],
                                             rhs=lf[psl, cs],
                                             start=True, stop=True)
                            # count: mask with accum (junk out tile)
                            mjk = dice.tile([128, 512], F32, tag="mjk")
                            nc.any.tensor_scalar(mjk, lbp, c_col, None,
                                                 AL.is_equal, AL.add,
                                                 accum_out=cntp[:, col])
                            # inter = mask * s_c ; accum -> inter partial
                            nc.vector.scalar_tensor_tensor(
                                scx[:, cs], lbp, c_col, scx[:, cs],
                                AL.is_equal, AL.mult, accum_out=intp[:, col])

                # reduce partials -> per (class, sample) -> loss row
                q3 = []
                for ptile in (scp, intp, cntp):
                    pq = pstiny.tile([4, 16], F32, space="PSUM", tag="tp")
                    nc.tensor.matmul(pq, lhsT=cls_grp, rhs=ptile,
                                     start=True, stop=True)
                    q = tiny.tile([4, 4], F32, tag="q44")
                    nc.vector.tensor_reduce(q, pq.rearrange("c (s h) -> c s h", s=4),
                                            axis=mybir.AxisListType.X, op=AL.add)
                    q3.append(q)
                ss4, int4, cnt4 = q3
                u = tiny.tile([4, 4], F32, tag="q44")
                nc.vector.scalar_tensor_tensor(u, ss4, 1e-5, cnt4, AL.add, AL.add)
                iu = tiny.tile([4, 4], F32, tag="q44")
                nc.vector.reciprocal(iu, u)
                d4 = tiny.tile([4, 4], F32, tag="q44")
                nc.vector.scalar_tensor_tensor(d4, int4, 2.0, iu, AL.mult, AL.mult)
                lsump = pstiny.tile([1, 4], F32, space="PSUM", tag="tp")
                nc.tensor.matmul(lsump, lhsT=ones64[0:4, 0:1], rhs=d4,
                                 start=True, stop=True)
                loss = tiny.tile([1, 4], F32, tag="row4")
                nc.vector.tensor_scalar(loss, lsump, -1.0 / NCLS, 1.0,
                                        AL.mult, AL.add)
                trow = tiny.tile([1, 4], F32, tag="row4")
                nc.vector.tensor_scalar(trow, loss, -16.0 / AGE_DIV, 16.0,
                                        AL.mult, AL.add)
                tneg = tiny.tile([1, 4], F32, tag="row4")
                nc.vector.tensor_scalar(tneg, trow, -1.0, None, AL.mult)
                ta = tiny.tile([1, 4], F32, tag="row4")
                nc.vector.tensor_tensor(ta, trow, tneg, op=AL.max)
                mrow = tiny.tile([1, 4], F32, tag="row4")
                nc.vector.tensor_scalar(mrow, ta, 16.0, None, AL.min)
                # sg01 = [loss < 1]  (1 when the self-paced branch flips sort)
                sgrow = tiny.tile([1, 4], F32, tag="row4")
                nc.vector.tensor_scalar(sgrow, loss, 1.0, None, AL.is_lt)

                # -------- keys: conf patch sums + sign-free ranks --------
                cfin = chin.tile([128, 128 * 32], F32, tag="cin")
                for si in range(GS):
                    row = slice(32 * si, 32 * si + 32)
                    nc.sync.dma_start(cfin[row, 0:2048],
                                      patch_view(ocnf_d[:])[s0 + si])
                    nc.sync.dma_start(cfin[row, 2048:4096],
                                      patch_view(acnf_d[:])[s0 + si])
                cfins.append(cfin)
                colred = keys.tile([128, 128], F32, tag="colred")
                nc.vector.tensor_reduce(colred,
                                        cfin[:].rearrange("p (n d) -> p n d", d=32),
                                        axis=mybir.AxisListType.X, op=AL.add)
                st1p = pstiny.tile([16, 128], F32, space="PSUM", tag="tp")
                nc.tensor.matmul(st1p, lhsT=rg8, rhs=colred, start=True, stop=True)
                st1 = keys.tile([16, 128], F32, tag="st1")
                nc.vector.tensor_copy(st1, st1p)
                ksump = pstiny.tile([4, 128], F32, space="PSUM", tag="tp")
                nc.tensor.matmul(ksump, lhsT=rg4, rhs=st1, start=True, stop=True)
                kT = keys.tile([4, 128], F32, tag="kT")
                nc.vector.tensor_copy(kT, ksump)
                kcp = pstiny.tile([128, 4], F32, space="PSUM", tag="tp")
                nc.tensor.transpose(kcp, kT, id4)
                kcols = keys.tile([128, 4], F32, tag="kcols")
                nc.vector.tensor_copy(kcols, kcp)

                # block-diagonal expansion: rhs[k,(si,p)] = kT[k,p]*[si==k]
                blk3 = blkmask.rearrange("k (s p) -> k s p", s=4)
                kTeo = keys.tile([4, 256], F32, tag="kTe")
                nc.vector.tensor_tensor(
                    kTeo[:].rearrange("k (s p) -> k s p", s=4),
                    _bcast_free(kT[0:4, 0:64], [(1, 4)]), blk3, op=AL.mult)
                kTea = keys.tile([4, 256], F32, tag="kTe")
                nc.vector.tensor_tensor(
                    kTea[:].rearrange("k (s p) -> k s p", s=4),
                    _bcast_free(kT[0:4, 64:128], [(1, 4)]), blk3, op=AL.mult)
                obc = ps256.tile([64, 256], F32, space="PSUM", tag="k2")
                abc = ps256.tile([64, 256], F32, space="PSUM", tag="k2")
                nc.tensor.matmul(obc, lhsT=ones64[0:4, :], rhs=kTeo,
                                 start=True, stop=True)
                nc.tensor.matmul(abc, lhsT=ones64[0:4, :], rhs=kTea,
                                 start=True, stop=True)
                # A+[q,(s,p)] = [raw_key_s[p] > raw_key_s[q]]
                Ao = keys.tile([64, 256], F32, tag="Ao")
                nc.vector.tensor_tensor(
                    Ao, obc, _bcast_free(kcols[0:64, :], [(2, 64)]), op=AL.is_gt)
                Aa = keys.tile([64, 256], F32, tag="Aa")
                nc.vector.tensor_tensor(
                    Aa, abc, _bcast_free(kcols[64:128, :], [(2, 64)]), op=AL.is_gt)
                # ascending ranks of raw sums
                rorp = ps256.tile([64, 256], F32, space="PSUM", tag="k2")
                racp = pstiny.tile([64, 4], F32, space="PSUM", tag="tp")
                for si in range(GS):
                    osl = slice(64 * si, 64 * si + 64)
                    nc.tensor.matmul(rorp[:, osl], lhsT=ones64, rhs=Ao[:, osl],
                                     start=True, stop=True)
                    nc.tensor.matmul(racp[:, si:si + 1], lhsT=Aa[:, osl],
                                     rhs=ones64[:, 0:1], start=True, stop=True)
                rors = keys.tile([64, 256], F32, tag="rors")
                nc.vector.tensor_copy(rors, rorp)
                racs = keys.tile([64, 4], F32, tag="racs")
                nc.vector.tensor_copy(racs, racp)

                # -------- fixup with sign + build gather index --------
                # sg broadcast [64, (s)] and [64, (s, p)]
                sgp = pstiny.tile([64, 4], F32, space="PSUM", tag="tp")
                nc.tensor.matmul(sgp, lhsT=ones64[0:1, :], rhs=sgrow,
                                 start=True, stop=True)
                sgb = keys.tile([64, 4], F32, tag="sgb")
                nc.vector.tensor_copy(sgb, sgp)
                # rank_o = r+ + sg*(63 - 2 r+)   (descending when sg=1)
                rob = keys.tile([64, 256], F32, tag="rob")
                nc.vector.tensor_scalar(rob, rors, -2.0, 63.0, AL.mult, AL.add)
                nc.vector.tensor_tensor(
                    rob, rob, _bcast_free(sgb[:, :], [(2, 64)]), op=AL.mult)
                nc.vector.tensor_tensor(rob, rob, rors, op=AL.add)
                # rank_a = r_a+ + (1-sg)*(63 - 2 r_a+)
                rac2 = keys.tile([64, 4], F32, tag="rac2")
                nc.vector.tensor_scalar(rac2, racs, -2.0, 63.0, AL.mult, AL.add)
                sga = keys.tile([64, 4], F32, tag="sga")
                nc.vector.tensor_scalar(sga, sgb, -1.0, 1.0, AL.mult, AL.add)
                nc.vector.tensor_tensor(rac2, rac2, sga, op=AL.mult)
                nc.vector.tensor_tensor(rac2, rac2, racs, op=AL.add)

                # M2[q,(s,p)] = [rank_a[q] == rank_o[p]] ; src = q @ M2
                M2 = keys.tile([64, 256], F32, tag="M2")
                nc.vector.tensor_tensor(
                    M2, rob, _bcast_free(rac2[:, :], [(2, 64)]), op=AL.is_equal)
                srcp = pstiny.tile([1, 256], F32, space="PSUM", tag="tp")
                nc.tensor.matmul(srcp, lhsT=qiota, rhs=M2, start=True, stop=True)

                rp1 = keys.tile([1, 256], F32, tag="rowt")
                nc.vector.tensor_scalar(rp1, rob[0:1, :], 1.0, None, AL.add)
                repl = keys.tile([1, 256], F32, tag="rowt")
                nc.vector.tensor_tensor(
                    repl, rp1, _bcast_free(mrow[0:1, :], [(2, 64)]), op=AL.is_le)
                x1 = keys.tile([1, 256], F32, tag="rowt")
                pio = _bcast_free(piota, [(1, 4)])
                nc.vector.tensor_tensor(x1, srcp, pio, op=AL.subtract)
                nc.vector.tensor_scalar(x1, x1, 64.0, None, AL.add)
                nc.vector.tensor_tensor(x1, x1, repl, op=AL.mult)
                nc.vector.tensor_tensor(x1, x1, pio, op=AL.add)

                sicp = pstiny.tile([64, 4], F32, space="PSUM", tag="tp")
                for si in range(GS):
                    nc.tensor.transpose(sicp[:, si:si + 1],
                                        x1[0:1, 64 * si:64 * si + 64], id1)
                sics = keys.tile([64, 4], F32, tag="sics")
                nc.vector.tensor_copy(sics, sicp)
                # w1g[p, 4s+phi] = sidx_s[p] * [p//16 == phi]
                w1g = keys.tile([64, 16], F32, tag="w1g")
                nc.vector.tensor_tensor(
                    w1g[:].rearrange("p (s f) -> p s f", s=4),
                    _bcast_free(sics[:, :], [(2, 4)]),
                    wmask4.rearrange("p (s f) -> p s f", s=4), op=AL.mult)
                # wig[pi, 4s+phi] = sidx_s[16 phi + pi]
                wig = pstiny.tile([16, 16], F32, space="PSUM", tag="tp")
                nc.tensor.matmul(wig, lhsT=wsel, rhs=w1g, start=True, stop=True)
                wigs = keys.tile([16, 16], F32, tag="wigs")
                nc.vector.tensor_copy(wigs, wig)
                # replicate rows to all 128 partitions, select own sample block
                wbig = ps256.tile([128, 16], F32, space="PSUM", tag="k2")
                nc.tensor.matmul(wbig, lhsT=pisel, rhs=wigs, start=True, stop=True)
                wsl = keys.tile([128, 16], F32, tag="wsl")
                nc.vector.tensor_tensor(wsl, wbig, smask, op=AL.mult)
                idxf = keys.tile([128, 4], F32, tag="idxf")
                ws_v = bass.AP(wsl[:].tensor, wsl[:].offset,
                               [list(wsl[:].ap[0]), [1, 4], [4, 4]])
                nc.vector.tensor_reduce(idxf, ws_v,
                                        axis=mybir.AxisListType.X, op=AL.add)
                idxt = keys.tile([128, 4], I16, tag="idxt")
                nc.vector.tensor_copy(idxt, idxf)
                idxts.append(idxt)

                if dbg_out:
                    for nm, t in [("dbg_loss", loss), ("dbg_m", mrow),
                                  ("dbg_sg", sgrow), ("dbg_rp1", rp1),
                                  ("dbg_repl", repl), ("dbg_sidx", x1),
                                  ("dbg_racs", rac2), ("dbg_idxf", idxf)]:
                        nc.sync.dma_start(dbg_d[nm][g], t)

            # ---------------- gather + write back, all chunks ----------------
            def load_chunk(g, osrcv, asrcv, dt):
                s0 = g * GS
                cin = chin.tile([128, 128 * 32], dt, tag="cin")
                for si in range(GS):
                    row = slice(32 * si, 32 * si + 32)
                    nc.sync.dma_start(cin[row, 0:2048], osrcv[s0 + si])
                    nc.sync.dma_start(cin[row, 2048:4096], asrcv[s0 + si])
                return cin

            def gather_store(g, cin, dstv, dt):
                s0 = g * GS
                cout = chout.tile([128, 64 * 32], dt, tag="cout")
                nc.gpsimd.ap_gather(
                    cout[:].rearrange("p (n d) -> p n d", d=32),
                    cin[:].rearrange("p (n d) -> p n d", d=32),
                    idxts[g][:],
                    channels=128, num_elems=128, d=32, num_idxs=64)
                for si in range(GS):
                    nc.sync.dma_start(
                        dstv[s0 + si],
                        cout[32 * si:32 * si + 32, :].rearrange(
                            "r (bi f) -> r bi f", bi=NH))

            chunks = [
                (patch_view(oimg_d[:, 0]), patch_view(aimg_d[:, 0]),
                 patch_view(out_img[:, 0]), F32),
                (patch_view(oimg_d[:, 1]), patch_view(aimg_d[:, 1]),
                 patch_view(out_img[:, 1]), F32),
                (patch_view(oimg_d[:, 2]), patch_view(aimg_d[:, 2]),
                 patch_view(out_img[:, 2]), F32),
                (patch_view(olab_d[:]), patch_view(alab_d[:]),
                 patch_view(out_lab[:]), I32),
            ]
            # stream: conf gathers first (tiles already resident), then the
            # 4 remaining chunks per group, loads running ahead of gathers
            pend = []
            for g in range(NG):
                for (ov, av, dv, dt) in chunks:
                    pend.append((g, load_chunk(g, ov, av, dt), dv, dt))
            for g in range(NG):
                gather_store(g, cfins[g], patch_view(out_cnf[:]), F32)
            for (g, cin, dv, dt) in pend:
                gather_store(g, cin, dv, dt)

    nc.compile()
    return nc


_cached = {}


def kernel(oimage, aimage, olabel, alabel, oconf, aconf, prediction,
           cur_step=None):
    if "nc" not in _cached:
        _cached["nc"] = build_core_kernel(debug=False)
    nc = _cached["nc"]
    cst = _build_consts()
    in_maps = []
    for i in range(N_CORES):
        sl = slice(i * SPB, (i + 1) * SPB)
        in_maps.append({
            "pred": np.ascontiguousarray(prediction[sl], np.float32),
            "oimg": np.ascontiguousarray(oimage[sl], np.float32),
            "aimg": np.ascontiguousarray(aimage[sl], np.float32),
            "olab": np.ascontiguousarray(olabel[sl], np.int32),
            "alab": np.ascontiguousarray(alabel[sl], np.int32),
            "ocnf": np.ascontiguousarray(oconf[sl], np.float32),
            "acnf": np.ascontiguousarray(aconf[sl], np.float32),
            "consts": cst,
        })
    res = run_bass_kernel_spmd(nc, in_maps, core_ids=list(range(N_CORES)))
    _exec_info["exec_time_ns"] = res.exec_time_ns
    img = np.concatenate([res.results[i]["out_img"] for i in range(N_CORES)])
    lab = np.concatenate([res.results[i]["out_lab"] for i in range(N_CORES)])
    cnf = np.concatenate([res.results[i]["out_cnf"] for i in range(N_CORES)])
    return img, lab.astype(np.int32), cnf
